# revision 2
# baseline (speedup 1.0000x reference)
"""Distributed Trainium2 kernel for nn_Attention_18562848653411.

Reference model: fc_in -> LayerNorm -> 4 sequential "refinement heads"
(qkv matmul + gelu, scores=q@k^T/C, att=scores@v, softmax over channels,
proj + gelu, residual with head-0 output) -> fc_out + PoseEncoding.

Sharding (8 NeuronCores): core c handles batch b=c//2, sequence half
h=c%2 (1024 of 2048 rows). All weights replicated. Per head, each pair
of cores all-gathers the fp8 activations h^T (1 MB) so k/v can be
computed for the full sequence locally; q/scores/att/softmax/proj stay
row-local. Head 0 needs no collective at all: fc_in+LayerNorm run over
the full batch item locally (x is an input), with the own sequence half
rotated first so the graph stays SPMD-uniform. Activations are kept in
transposed [C, S] layout so every matmul consumes operands natively
(contraction dim on partitions); att^T is produced directly with
v-stationary matmuls and the channel softmax uses a ones-matmul
denominator + rank-1 broadcast (att <= ~10, so exp needs no max
subtraction).

Compute: fp8e4 (e4m3) matmuls with DoubleRow perf mode (2 fp8 weights
per PE cell -> 2 k-tiles per instruction) and f32 PSUM accumulation.
The channel softmax renormalizes away quantization noise each head and
the exact-f32 pose encoding dominates the output, so end-to-end rel err
stays ~3e-4 (measured vs the f32 reference; gate is 2e-2). The softmax
probabilities are stored scaled by 64 so they sit in fp8e4's normal
range; the proj activation applies the compensating 1/64 via its input
scale. exp() values (up to e^10) stay bf16.
"""

import numpy as np
import ml_dtypes

import concourse.bass as bass
import concourse.mybir as mybir
import concourse.tile as tile
from concourse import bacc
from concourse.bass_utils import run_bass_kernel_spmd

N_CORES = 8
PAIRS = [[0, 1], [2, 3], [4, 5], [6, 7]]
B, S, C = 4, 2048, 1024
H = 4
S_OWN = S // 2        # rows per core
T = S                 # full sequence (k/v length)
KT = C // 128         # contraction tiles

F32 = mybir.dt.float32
BF16 = mybir.dt.bfloat16
F8 = mybir.dt.float8e4
GELU = mybir.ActivationFunctionType.Gelu
EXP = mybir.ActivationFunctionType.Exp
SQRT = mybir.ActivationFunctionType.Sqrt
AX = mybir.AxisListType.X
SUB = mybir.AluOpType.subtract
MULT = mybir.AluOpType.mult
BYPASS = mybir.AluOpType.bypass
DROW = mybir.MatmulPerfMode.DoubleRow

NP_BF16 = ml_dtypes.bfloat16
NP_F8 = ml_dtypes.float8_e4m3

SM_SCALE = 64.0       # softmax store scale (fp8e4 normal range)


def build(n_heads: int = H, with_bias: bool = True, with_ln_affine: bool = True) -> bacc.Bacc:
    """Build the SPMD graph. n_heads>4 cycles weights (timing builds).
    with_bias=False elides the K=1 bias matmuls (all harness biases are 0);
    with_ln_affine=False elides the LayerNorm gain/bias application."""
    nc = bacc.Bacc(num_devices=N_CORES, name="attn")

    # x transposed, full batch item, OWN sequence half first (so the graph is
    # SPMD-uniform: "own" rows are always columns 0:S_OWN)
    x_t = nc.dram_tensor("x_t", [C, T], F8, kind="ExternalInput")
    fcw = nc.dram_tensor("fc_in_wT", [C, C], F8, kind="ExternalInput")
    fcb = nc.dram_tensor("fc_in_b_row", [1, C], BF16, kind="ExternalInput")
    lng = nc.dram_tensor("ln_g_row", [1, C], F32, kind="ExternalInput")
    lnb = nc.dram_tensor("ln_b_row", [1, C], F32, kind="ExternalInput")
    qkw = nc.dram_tensor("qk_w_tiled", [H, 16, 128, KT, 128], F8, kind="ExternalInput")
    vw = nc.dram_tensor("v_wT", [H, C, C], F8, kind="ExternalInput")
    qb = nc.dram_tensor("q_b_col", [H, 128, 8], F32, kind="ExternalInput")
    kb = nc.dram_tensor("k_b_col", [H, 128, 8], F32, kind="ExternalInput")
    vb = nc.dram_tensor("v_b_row", [H, 1, C], BF16, kind="ExternalInput")
    pw = nc.dram_tensor("proj_w_tiled", [H, 8, 128, KT, 128], F8, kind="ExternalInput")
    pb = nc.dram_tensor("proj_b_col", [H, 128, 8], F32, kind="ExternalInput")
    fow = nc.dram_tensor("fc_out_wT", [C, C], F8, kind="ExternalInput")
    fob = nc.dram_tensor("fc_out_b_row", [1, C], BF16, kind="ExternalInput")
    pe = nc.dram_tensor("pe", [S_OWN, C], F32, kind="ExternalInput")
    out = nc.dram_tensor("out", [S_OWN, C], F32, kind="ExternalOutput")

    def mm2(ps, lhsT, rhs, start, stop):
        nc.tensor.matmul(ps, lhsT, rhs, start=start, stop=stop, perf_mode=DROW)

    with tile.TileContext(nc) as tc:
        with (
            tc.tile_pool(name="dram", bufs=1, space="DRAM") as dram,
            tc.tile_pool(name="pers", bufs=1) as pers,
            tc.tile_pool(name="hown", bufs=2) as hown_pool,
            tc.tile_pool(name="psA", bufs=2, space="PSUM") as psA,
            tc.tile_pool(name="psB", bufs=2, space="PSUM") as psB,
            tc.tile_pool(name="small", bufs=2) as small,
            tc.tile_pool(name="tmp", bufs=2) as tmp,
        ):
            SH2 = S_OWN // 2
            bounce_in = [[dram.tile([C, SH2], F8, name=f"agin{i}_{ch}")
                          for ch in range(2)] for i in range(1, n_heads)]
            bounce_in.insert(0, None)
            bounce_out = [[dram.tile([2 * C, SH2], F8, name=f"agout{i}_{ch}")
                           for ch in range(2)] for i in range(1, n_heads)]
            bounce_out.insert(0, None)
            # head 0 needs no collective: fc_in+LN is computed for the FULL
            # batch item locally; the partner half h0^T stays in SBUF

            ones_bf = pers.tile([1, 128], BF16)
            nc.vector.memset(ones_bf[:], 1.0)
            # 1/SM_SCALE so the softmax reciprocal comes out pre-scaled
            ones_col = pers.tile([128, 1], BF16)
            nc.vector.memset(ones_col[:], 1.0 / SM_SCALE)
            eps_t = pers.tile([128, 1], F32)
            nc.vector.memset(eps_t[:], 1e-5)
            pred = pers.tile([128, KT, S_OWN], F8)
            h0p_sb = pers.tile([128, KT, S_OWN], F8, name="h0p_sb")

            def all_gather(idx, h_src, ch):
                """Gather the s-half [512*ch, 512*(ch+1)) of h^T across the pair."""
                for cc in range(KT):
                    nc.sync.dma_start(bounce_in[idx][ch][cc * 128:(cc + 1) * 128, :],
                                      h_src[:, cc, ch * SH2:(ch + 1) * SH2])
                nc.gpsimd.collective_compute(
                    "AllGather", BYPASS, replica_groups=PAIRS,
                    ins=[bounce_in[idx][ch][:].opt()], outs=[bounce_out[idx][ch][:].opt()],
                )

            # -------- stage 0: fc_in + LayerNorm over the FULL batch item ------
            h_own = hown_pool.tile([128, KT, S_OWN], F8, tag="hown", name="hT0")
            with tc.tile_pool(name="stage0", bufs=1) as s0:
                x_sb = s0.tile([128, KT, T], F8)
                fcw_sb = s0.tile([128, KT, C], F8)
                for kk in range(KT):
                    nc.sync.dma_start(x_sb[:, kk, :], x_t[kk * 128:(kk + 1) * 128, :])
                    nc.sync.dma_start(fcw_sb[:, kk, :], fcw[kk * 128:(kk + 1) * 128, :])
                if with_bias:
                    fcb_sb = s0.tile([1, C], BF16)
                    nc.sync.dma_start(fcb_sb[:], fcb[:])
                if with_ln_affine:
                    g_bc = s0.tile([128, C], F32)
                    nc.sync.dma_start(g_bc[:], bass.AP(tensor=lng, offset=0,
                                                       ap=[[0, 128], [1, C]]))
                    b_bc = s0.tile([128, C], F32)
                    nc.sync.dma_start(b_bc[:], bass.AP(tensor=lnb, offset=0,
                                                       ap=[[0, 128], [1, C]]))

                for ss in range(16):
                    ps = psA.tile([128, C], F32, tag="mmA")
                    for kk in range(0, KT, 2):
                        for nch in range(2):
                            nsl = slice(nch * 512, (nch + 1) * 512)
                            mm2(ps[:, nsl], x_sb[:, kk:kk + 2, ss * 128:(ss + 1) * 128],
                                fcw_sb[:, kk:kk + 2, nsl], start=(kk == 0),
                                stop=(not with_bias and kk == KT - 2))
                    if with_bias:
                        for nch in range(2):
                            nsl = slice(nch * 512, (nch + 1) * 512)
                            nc.tensor.matmul(ps[:, nsl], ones_bf[:], fcb_sb[0:1, nsl],
                                             start=False, stop=True)
                    stats = small.tile([128, 2, 6], F32, tag="bnst")
                    nc.vector.bn_stats(stats[:, 0, :], ps[:, 0:512])
                    nc.vector.bn_stats(stats[:, 1, :], ps[:, 512:1024])
                    mv = small.tile([128, 2], F32, tag="mv")
                    nc.vector.bn_aggr(mv[:], stats[:])
                    rstd = small.tile([128, 1], F32, tag="rstd")
                    nc.scalar.activation(rstd[:], mv[:, 1:2], SQRT, bias=eps_t[:], scale=1.0)
                    nc.vector.reciprocal(rstd[:], rstd[:])
                    hnb = s0.tile([128, C], BF16, tag="hnb", bufs=2)
                    if with_ln_affine:
                        hn = s0.tile([128, C], F32, tag="hn", bufs=2)
                        nc.vector.tensor_scalar(hn[:], ps[:], mv[:, 0:1], rstd[:],
                                                op0=SUB, op1=MULT)
                        nc.vector.tensor_mul(hn[:], hn[:], g_bc[:])
                        nc.vector.tensor_add(hnb[:], hn[:], b_bc[:])
                    else:
                        # (x - mu) * rstd on the vector engine
                        nc.vector.tensor_scalar(hnb[:], ps[:], mv[:, 0:1], rstd[:],
                                                op0=SUB, op1=MULT)
                    # bf16 transpose staging (DMA transpose needs 2-byte), then
                    # one DVE pass converts the s-block to fp8
                    h_stg = s0.tile([128, KT, 128], BF16, tag="hstg", bufs=3)
                    for cc in range(KT):
                        nc.sync.dma_start(h_stg[:, cc, :],
                                          hnb[:, cc * 128:(cc + 1) * 128], transpose=True)
                    dst = h_own if ss < 8 else h0p_sb
                    sb = (ss % 8) * 128
                    nc.vector.tensor_copy(dst[:, :, sb:sb + 128], h_stg[:])

            # ---------------- heads ----------------
            for i in range(n_heads):
                wi = i % H  # weight index (cycling for timing builds)
                with (
                    tc.tile_pool(name=f"head{i}", bufs=1) as hp,
                    tc.tile_pool(name=f"wstr{i}", bufs=5 if not with_bias else 3) as wstr,
                    tc.tile_pool(name=f"hh{i}", bufs=1) as hhp,
                    tc.tile_pool(name=f"attT{i}", bufs=1) as attp,
                ):
                    q_sb = hp.tile([128, 8, S_OWN], F8, name="q_sb")
                    k_sb = hp.tile([128, 8, T], F8, name="k_sb")
                    v_sb = hp.tile([128, 16, C], F8, name="v_sb")
                    sc_sb = hp.tile([128, 16, 256], F8, name="sc_sb")
                    wv_sb = hp.tile([128, KT, C], F8, name="wv_sb")
                    nc.sync.dma_start(wv_sb[:], vw[wi].rearrange("(k p) n -> p k n", p=128))
                    if with_bias:
                        vb_sb = small.tile([1, C], BF16, tag="vb", bufs=1)
                        nc.sync.dma_start(vb_sb[:], vb[wi])
                    qb_sb = small.tile([128, 8], F32, tag="qb")
                    nc.sync.dma_start(qb_sb[:], qb[wi])
                    kb_sb = small.tile([128, 8], F32, tag="kb")
                    nc.sync.dma_start(kb_sb[:], kb[wi])
                    pb_sb = small.tile([128, 8], F32, tag="pb")
                    nc.sync.dma_start(pb_sb[:], pb[wi])

                    # ---- q^T (no dependency on the all-gather)
                    for co in range(8):
                        wq = wstr.tile([128, KT, 128], F8, tag="wq")
                        nc.sync.dma_start(wq[:], qkw[wi, co])
                        ps = psA.tile([128, C], F32, tag="mmA")
                        for kk in range(0, KT, 2):
                            for nch in range(2):
                                nsl = slice(nch * 512, (nch + 1) * 512)
                                mm2(ps[:, nsl], wq[:, kk:kk + 2, :],
                                    h_own[:, kk:kk + 2, nsl],
                                    start=(kk == 0), stop=(kk == KT - 2))
                        nc.scalar.activation(q_sb[:, co, :], ps[:], GELU,
                                             bias=qb_sb[:, co:co + 1], scale=1.0)

                    # ---- k^T and v over the gathered sequence, per t-quarter.
                    # Quarters ordered so the first two depend only on AG chunk 0.
                    # Head 0: own-half quarters read h_own directly; partner
                    # half comes from the local h0p staging (no collective).
                    quarters = (((0, 0), (0, 1), (1, 0), (1, 1)) if i == 0 else
                                ((0, 0), (1, 0), (0, 1), (1, 1)))
                    for rk, sloc in quarters:
                        tbase = rk * 1024 + sloc * 512
                        if i == 0:
                            hsrc = h_own if rk == 0 else h0p_sb
                            hh = hsrc[:, :, sloc * 512:(sloc + 1) * 512]
                        else:
                            hh = hhp.tile([128, KT, 512], F8, tag="hh", name="hh")
                            for kk in range(KT):
                                nc.sync.dma_start(
                                    hh[:, kk, :],
                                    bounce_out[i][sloc][rk * C + kk * 128:
                                                        rk * C + (kk + 1) * 128, :])
                        for co in range(8):
                            wk = wstr.tile([128, KT, 128], F8, tag="wq")
                            nc.sync.dma_start(wk[:], qkw[wi, 8 + co])
                            ps = psB.tile([128, 512], F32, tag="mmB", bufs=3)
                            for kk in range(0, KT, 2):
                                mm2(ps[:], wk[:, kk:kk + 2, :], hh[:, kk:kk + 2, :],
                                    start=(kk == 0), stop=(kk == KT - 2))
                            nc.scalar.activation(
                                k_sb[:, co, tbase:tbase + 512],
                                ps[:], GELU, bias=kb_sb[:, co:co + 1], scale=1.0)
                        for tt in range(4):
                            ps = psA.tile([128, C], F32, tag="mmA")
                            for kk in range(0, KT, 2):
                                for nch in range(2):
                                    nsl = slice(nch * 512, (nch + 1) * 512)
                                    mm2(ps[:, nsl], hh[:, kk:kk + 2, tt * 128:(tt + 1) * 128],
                                        wv_sb[:, kk:kk + 2, nsl], start=(kk == 0),
                                        stop=(not with_bias and kk == KT - 2))
                            if with_bias:
                                for nch in range(2):
                                    nsl = slice(nch * 512, (nch + 1) * 512)
                                    nc.tensor.matmul(ps[:, nsl], ones_bf[:], vb_sb[0:1, nsl],
                                                     start=False, stop=True)
                            nc.scalar.activation(v_sb[:, tbase // 128 + tt, :], ps[:], GELU)

                    # ---- scores^T, att^T (direct, v-stationary), softmax via
                    # ones-matmul denominator (att <= ~10 so exp needs no max-sub);
                    # proj per s-half, interleaved so each AG chunk launches early
                    h_new = hown_pool.tile([128, KT, S_OWN], F8, tag="hown", name=f"hT{i + 1}")
                    attsmT = attp.tile([128, KT, S_OWN], BF16, tag="attT", name="attsmT")
                    attn8 = attp.tile([128, KT, S_OWN], F8, tag="attn8", name="attn8")

                    def score_att_quarter(sq):
                        ssl = slice(sq * 256, (sq + 1) * 256)
                        for tt in range(16):
                            ps = psB.tile([128, 512], F32, tag="mmB", bufs=3, name="ps")
                            for cc in range(0, 8, 2):
                                mm2(ps[:, 0:256], k_sb[:, cc:cc + 2, tt * 128:(tt + 1) * 128],
                                    q_sb[:, cc:cc + 2, ssl], start=(cc == 0), stop=(cc == 6))
                            nc.vector.tensor_scalar_mul(sc_sb[:, tt, :], ps[:, 0:256], 1.0 / C)
                        for co in range(8):
                            ps = psB.tile([128, 512], F32, tag="mmB", bufs=3, name="ps")
                            for tt in range(0, 16, 2):
                                mm2(ps[:, 0:256],
                                    v_sb[:, tt:tt + 2, co * 128:(co + 1) * 128],
                                    sc_sb[:, tt:tt + 2, :],
                                    start=(tt == 0), stop=(tt == 14))
                            nc.scalar.activation(attsmT[:, co, ssl], ps[:, 0:256], EXP)
                        dn = psB.tile([1, 256], F32, tag="denom", bufs=1, name="dn")
                        for co in range(8):
                            nc.tensor.matmul(dn[:], ones_col[:], attsmT[:, co, ssl],
                                             start=(co == 0), stop=(co == 7))
                        rr = small.tile([1, 256], F32, tag="rr", name="rr")
                        nc.vector.reciprocal(rr[:], dn[:])
                        rrb = small.tile([1, 256], BF16, tag="rrb", name="rrb")
                        nc.vector.tensor_copy(rrb[:], rr[:])
                        bc = psB.tile([128, 256], F32, tag="mmB", bufs=3, name="bc")
                        nc.tensor.matmul(bc[:], ones_bf[:], rrb[:], start=True, stop=True)
                        for co in range(8):
                            nc.vector.tensor_mul(attn8[:, co, ssl], attsmT[:, co, ssl], bc[:])

                    def proj_half(ch):
                        nsl = slice(ch * 512, (ch + 1) * 512)
                        for co in range(8):
                            wpp = wstr.tile([128, KT, 128], F8, tag="wq", name="wpp")
                            nc.sync.dma_start(wpp[:], pw[wi, co])
                            ps = psB.tile([128, 512], F32, tag="mmB", bufs=3, name="ps")
                            for cc in range(0, 8, 2):
                                mm2(ps[:], wpp[:, cc:cc + 2, :], attn8[:, cc:cc + 2, nsl],
                                    start=(cc == 0), stop=(cc == 6))
                            if i == 0:
                                nc.scalar.activation(h_new[:, co, nsl], ps[:], GELU,
                                                     bias=pb_sb[:, co:co + 1],
                                                     scale=1.0 / SM_SCALE)
                                nc.vector.tensor_copy(pred[:, co, nsl], h_new[:, co, nsl])
                            else:
                                gtmp = tmp.tile([128, 512], BF16, tag="gtmp", name="gtmp")
                                nc.scalar.activation(gtmp[:], ps[:], GELU,
                                                     bias=pb_sb[:, co:co + 1],
                                                     scale=1.0 / SM_SCALE)
                                nc.vector.tensor_add(h_new[:, co, nsl], gtmp[:],
                                                     pred[:, co, nsl])
                        if i < n_heads - 1:
                            all_gather(i + 1, h_new, ch)

                    score_att_quarter(0)
                    score_att_quarter(1)
                    proj_half(0)
                    score_att_quarter(2)
                    score_att_quarter(3)
                    proj_half(1)
                    h_own = h_new

            # ---------------- fc_out + pose encoding ----------------
            with (
                tc.tile_pool(name="fco", bufs=1) as fo,
                tc.tile_pool(name="fco2", bufs=2) as fo2,
            ):
                fow_sb = fo.tile([128, KT, C], F8)
                nc.sync.dma_start(fow_sb[:], fow[:].rearrange("(k p) n -> p k n", p=128))
                if with_bias:
                    fob_sb = fo.tile([1, C], BF16)
                    nc.sync.dma_start(fob_sb[:], fob[:])
                for ss in range(8):
                    ps = psA.tile([128, C], F32, tag="mmA")
                    for kk in range(0, KT, 2):
                        for nch in range(2):
                            nsl = slice(nch * 512, (nch + 1) * 512)
                            mm2(ps[:, nsl], h_own[:, kk:kk + 2, ss * 128:(ss + 1) * 128],
                                fow_sb[:, kk:kk + 2, nsl], start=(kk == 0),
                                stop=(not with_bias and kk == KT - 2))
                    if with_bias:
                        for nch in range(2):
                            nsl = slice(nch * 512, (nch + 1) * 512)
                            nc.tensor.matmul(ps[:, nsl], ones_bf[:], fob_sb[0:1, nsl],
                                             start=False, stop=True)
                    pe_sb = fo2.tile([128, C], F32, tag="pe")
                    nc.sync.dma_start(pe_sb[:], pe[ss * 128:(ss + 1) * 128, :])
                    o_sb = fo2.tile([128, C], F32, tag="osb")
                    nc.vector.tensor_add(o_sb[:], ps[:], pe_sb[:])
                    nc.sync.dma_start(out[ss * 128:(ss + 1) * 128, :], o_sb[:])

    nc.compile()
    return nc


def build_null() -> bacc.Bacc:
    """Same I/O signature, ~no compute: measures the dispatch floor."""
    nc = bacc.Bacc(num_devices=N_CORES, name="attn_null")
    nc.dram_tensor("x_t", [C, T], F8, kind="ExternalInput")
    nc.dram_tensor("fc_in_wT", [C, C], F8, kind="ExternalInput")
    nc.dram_tensor("fc_in_b_row", [1, C], BF16, kind="ExternalInput")
    nc.dram_tensor("ln_g_row", [1, C], F32, kind="ExternalInput")
    nc.dram_tensor("ln_b_row", [1, C], F32, kind="ExternalInput")
    nc.dram_tensor("qk_w_tiled", [H, 16, 128, KT, 128], F8, kind="ExternalInput")
    nc.dram_tensor("v_wT", [H, C, C], F8, kind="ExternalInput")
    nc.dram_tensor("q_b_col", [H, 128, 8], F32, kind="ExternalInput")
    nc.dram_tensor("k_b_col", [H, 128, 8], F32, kind="ExternalInput")
    nc.dram_tensor("v_b_row", [H, 1, C], BF16, kind="ExternalInput")
    nc.dram_tensor("proj_w_tiled", [H, 8, 128, KT, 128], F8, kind="ExternalInput")
    nc.dram_tensor("proj_b_col", [H, 128, 8], F32, kind="ExternalInput")
    nc.dram_tensor("fc_out_wT", [C, C], F8, kind="ExternalInput")
    nc.dram_tensor("fc_out_b_row", [1, C], BF16, kind="ExternalInput")
    pe = nc.dram_tensor("pe", [S_OWN, C], F32, kind="ExternalInput")
    out = nc.dram_tensor("out", [S_OWN, C], F32, kind="ExternalOutput")
    with tile.TileContext(nc) as tc:
        with tc.tile_pool(name="p", bufs=2) as p:
            for ss in range(8):
                t = p.tile([128, C], F32, tag="t")
                nc.sync.dma_start(t[:], pe[ss * 128:(ss + 1) * 128, :])
                nc.sync.dma_start(out[ss * 128:(ss + 1) * 128, :], t[:])
    nc.compile()
    return nc


def _pose_enc_np(s, f):
    pos = np.arange(s, dtype=np.float32)[:, None]
    div = (1.0 / (1000.0 ** (2.0 * np.arange(f, dtype=np.float32) / np.float32(f))))[None, :]
    p = np.zeros((s, f), np.float32)
    p[0::2, :] = np.sin(pos[0::2] * div)
    p[1::2, :] = np.cos(pos[1::2] * div)
    return p


def _bf(a):
    return np.ascontiguousarray(np.asarray(a, np.float32).astype(NP_BF16))


def _f8(a):
    return np.ascontiguousarray(np.asarray(a, np.float32).astype(NP_F8))


def _f32(a):
    return np.ascontiguousarray(np.asarray(a, np.float32))


def prepare_in_maps(x, fc_in_w, fc_in_b, ln_g, ln_b, qkv_w, qkv_b, proj_w, proj_b,
                    fc_out_w, fc_out_b):
    x = np.asarray(x, np.float32)
    qkv_w = np.asarray(qkv_w, np.float32)
    qkv_b = np.asarray(qkv_b, np.float32)
    proj_w = np.asarray(proj_w, np.float32)

    # [H, c_in, 2C] with q columns then k columns -> [H, 16, 128, KT, 128]
    qkT = np.concatenate([qkv_w[:, 0:C, :].transpose(0, 2, 1),
                          qkv_w[:, C:2 * C, :].transpose(0, 2, 1)], axis=2)
    qk_tiled = _f8(qkT.reshape(H, KT, 128, 16, 128).transpose(0, 3, 2, 1, 4))
    v_wT = _f8(qkv_w[:, 2 * C:, :].transpose(0, 2, 1))
    pw_tiled = _f8(proj_w.transpose(0, 2, 1).reshape(H, KT, 128, 8, 128).transpose(0, 3, 2, 1, 4))

    shared = {
        "fc_in_wT": _f8(np.asarray(fc_in_w, np.float32).T),
        "fc_in_b_row": _bf(np.asarray(fc_in_b)[None, :]),
        "ln_g_row": _f32(np.asarray(ln_g)[None, :]),
        "ln_b_row": _f32(np.asarray(ln_b)[None, :]),
        "qk_w_tiled": qk_tiled,
        "v_wT": v_wT,
        "q_b_col": _f32(qkv_b[:, 0:C].reshape(H, 8, 128).transpose(0, 2, 1)),
        "k_b_col": _f32(qkv_b[:, C:2 * C].reshape(H, 8, 128).transpose(0, 2, 1)),
        "v_b_row": _bf(qkv_b[:, 2 * C:][:, None, :]),
        "proj_w_tiled": pw_tiled,
        "proj_b_col": _f32(np.asarray(proj_b, np.float32).reshape(H, 8, 128).transpose(0, 2, 1)),
        "fc_out_wT": _f8(np.asarray(fc_out_w, np.float32).T),
        "fc_out_b_row": _bf(np.asarray(fc_out_b)[None, :]),
    }
    pe_full = _pose_enc_np(S, C)
    in_maps = []
    for core in range(N_CORES):
        b, half = divmod(core, 2)
        own = x[b, half * S_OWN:(half + 1) * S_OWN, :].T
        other = x[b, (1 - half) * S_OWN:(2 - half) * S_OWN, :].T
        m = dict(shared)
        m["x_t"] = _f8(np.concatenate([own, other], axis=1))
        m["pe"] = np.ascontiguousarray(pe_full[half * S_OWN:(half + 1) * S_OWN, :])
        in_maps.append(m)
    return in_maps


_NC_CACHE = {}


def get_nc(n_heads=H, with_bias=True, with_ln_affine=True):
    key = (n_heads, with_bias, with_ln_affine)
    if key not in _NC_CACHE:
        _NC_CACHE[key] = build(n_heads, with_bias, with_ln_affine)
    return _NC_CACHE[key]


_EXEC_CACHE = {}


def _get_executable(nc):
    """One jitted collectives executable per process (loading a second one
    hangs the axon worker); reused across kernel() calls."""
    key = id(nc)
    if key in _EXEC_CACHE:
        return _EXEC_CACHE[key]
    import jax
    from jax.sharding import Mesh, PartitionSpec, NamedSharding
    from jax.experimental.shard_map import shard_map
    from concourse import bass2jax
    import concourse.mybir as mybir_

    bass2jax.install_neuronx_cc_hook()
    partition_name = nc.partition_id_tensor.name if nc.partition_id_tensor else None
    in_names, out_names, out_avals, zero_outs = [], [], [], []
    for alloc in nc.m.functions[0].allocations:
        if not isinstance(alloc, mybir_.MemoryLocationSet):
            continue
        name = alloc.memorylocations[0].name
        if alloc.kind == "ExternalInput":
            if name != partition_name:
                in_names.append(name)
        elif alloc.kind == "ExternalOutput":
            out_names.append(name)
            shape = tuple(alloc.tensor_shape)
            dtype = mybir_.dt.np(alloc.dtype)
            out_avals.append(jax.core.ShapedArray(shape, dtype))
            zero_outs.append(np.zeros(shape, dtype))
    n_params = len(in_names)
    n_outs = len(out_avals)
    all_in = in_names + out_names + ([partition_name] if partition_name else [])
    donate = tuple(range(n_params, n_params + n_outs))

    def _body(*args):
        operands = list(args)
        if partition_name is not None:
            operands.append(bass2jax.partition_id_tensor())
        return tuple(bass2jax._bass_exec_p.bind(
            *operands, out_avals=tuple(out_avals), in_names=tuple(all_in),
            out_names=tuple(out_names), lowering_input_output_aliases=(),
            sim_require_finite=True, sim_require_nnan=True, nc=nc))

    devices = jax.devices()[:N_CORES]
    mesh = Mesh(np.asarray(devices), ("core",))
    sharded = jax.jit(
        shard_map(_body, mesh=mesh,
                  in_specs=(PartitionSpec("core"),) * (n_params + n_outs),
                  out_specs=(PartitionSpec("core"),) * len(out_names),
                  check_rep=False),
        donate_argnums=donate, keep_unused=True)
    sh = NamedSharding(mesh, PartitionSpec("core"))
    entry = (sharded, sh, in_names[:n_params], out_names, out_avals, zero_outs)
    _EXEC_CACHE[key] = entry
    return entry


def flags_for(inputs):
    with_bias = not (np.all(np.asarray(inputs["fc_in_b"]) == 0)
                     and np.all(np.asarray(inputs["qkv_b"]) == 0)
                     and np.all(np.asarray(inputs["proj_b"]) == 0)
                     and np.all(np.asarray(inputs["fc_out_b"]) == 0))
    with_ln = not (np.all(np.asarray(inputs["ln_g"]) == 1)
                   and np.all(np.asarray(inputs["ln_b"]) == 0))
    return with_bias, with_ln


def kernel(**inputs) -> np.ndarray:
    with_bias = not (np.all(np.asarray(inputs["fc_in_b"]) == 0)
                     and np.all(np.asarray(inputs["qkv_b"]) == 0)
                     and np.all(np.asarray(inputs["proj_b"]) == 0)
                     and np.all(np.asarray(inputs["fc_out_b"]) == 0))
    with_ln = not (np.all(np.asarray(inputs["ln_g"]) == 1)
                   and np.all(np.asarray(inputs["ln_b"]) == 0))
    nc = get_nc(H, with_bias, with_ln)
    in_maps = prepare_in_maps(**inputs)
    import jax
    sharded, sh, in_names, out_names, out_avals, zero_outs = _get_executable(nc)
    concat_in = [jax.device_put(
        np.concatenate([np.asarray(in_maps[c][nm]) for c in range(N_CORES)], axis=0), sh)
        for nm in in_names]
    concat_zeros = [jax.device_put(
        np.zeros((N_CORES * z.shape[0], *z.shape[1:]), z.dtype), sh) for z in zero_outs]
    out_arrs = sharded(*concat_in, *concat_zeros)
    jax.block_until_ready(out_arrs)
    oi = out_names.index("out")
    per_core = np.asarray(out_arrs[oi]).reshape(N_CORES, *out_avals[oi].shape)
    out_full = np.empty((B, S, C), np.float32)
    for core in range(N_CORES):
        b, half = divmod(core, 2)
        out_full[b, half * S_OWN:(half + 1) * S_OWN, :] = per_core[core]
    return out_full


# revision 3
# speedup vs baseline: 1.2441x; 1.2441x over previous
"""Distributed Trainium2 kernel for nn_Attention_18562848653411.

Reference model: fc_in -> LayerNorm -> 4 sequential "refinement heads"
(qkv matmul + gelu, scores=q@k^T/C, att=scores@v, softmax over channels,
proj + gelu, residual with head-0 output) -> fc_out + PoseEncoding.

Sharding (8 NeuronCores): core c handles batch b=c//2, sequence half
h=c%2 (1024 of 2048 rows). All weights replicated; every stage is
row-local except k/v, which need h for the full sequence.

Pair exchange runs as a ReduceScatter sum-trick instead of AllGather:
each core sends its h^T chunk DUPLICATED ([h; h], so the collective is
rank-symmetric), receives sum = h_own + h_partner, and recovers the
partner half with one subtract (on the otherwise-idle GpSimd engine).
This keeps the graph SPMD-uniform with the partner data at a fixed
t-slot, moves half the collective bytes of an AllGather, and lets all
own-half work (q, k/v over own rows, own-t scores) run before the
collective lands. fc_in+LayerNorm is computed for own rows only and
head 0 consumes the same exchange as every other head.

Activations are kept in transposed [C, S] layout so every matmul
consumes operands natively (contraction on partitions); att^T is
produced directly with v-stationary matmuls and the channel softmax
uses a ones-matmul denominator + rank-1 broadcast (att <= ~10, so exp
needs no max subtraction).

Compute: fp8e4 (e4m3) matmuls with DoubleRow perf mode (2 fp8 weights
per PE cell -> 2 k-tiles per instruction) and f32 PSUM accumulation.
The channel softmax renormalizes away quantization noise each head and
the exact-f32 pose encoding dominates the output, so end-to-end rel
err stays ~3.5e-4 (measured vs the f32 reference; gate is 2e-2). The
softmax probabilities are stored scaled by 64 so they sit in fp8e4's
normal range; the proj activation applies the compensating 1/64 via
its input scale. exp() values (up to e^10) stay bf16. DMAs are batched
into few large multi-dim transfers (the DGE costs ~625ns per
instruction regardless of size).
"""

import numpy as np
import ml_dtypes

import concourse.bass as bass
import concourse.mybir as mybir
import concourse.tile as tile
from concourse import bacc
from concourse.bass_utils import run_bass_kernel_spmd

N_CORES = 8
PAIRS = [[0, 1], [2, 3], [4, 5], [6, 7]]
B, S, C = 4, 2048, 1024
H = 4
S_OWN = S // 2        # rows per core
T = S                 # full sequence (k/v length)
KT = C // 128         # contraction tiles
SH2 = S_OWN // 2      # exchange chunk (s columns)

F32 = mybir.dt.float32
BF16 = mybir.dt.bfloat16
F8 = mybir.dt.float8e4
GELU = mybir.ActivationFunctionType.Gelu
EXP = mybir.ActivationFunctionType.Exp
SQRT = mybir.ActivationFunctionType.Sqrt
IDENT = mybir.ActivationFunctionType.Identity
SUB = mybir.AluOpType.subtract
MULT = mybir.AluOpType.mult
ADD = mybir.AluOpType.add
BYPASS = mybir.AluOpType.bypass
DROW = mybir.MatmulPerfMode.DoubleRow

NP_BF16 = ml_dtypes.bfloat16
NP_F8 = ml_dtypes.float8_e4m3

SM_SCALE = 64.0       # softmax store scale (fp8e4 normal range)


def build(n_heads: int = H, with_bias: bool = True, with_ln_affine: bool = True) -> bacc.Bacc:
    """Build the SPMD graph. n_heads>4 cycles weights (timing builds).
    with_bias=False elides the K=1 bias matmuls (all harness biases are 0);
    with_ln_affine=False elides the LayerNorm gain/bias application."""
    nc = bacc.Bacc(num_devices=N_CORES, name="attn")

    x_t = nc.dram_tensor("x_t", [C, S_OWN], F8, kind="ExternalInput")
    fcw = nc.dram_tensor("fc_in_wT", [C, C], F8, kind="ExternalInput")
    fcb = nc.dram_tensor("fc_in_b_row", [1, C], BF16, kind="ExternalInput")
    lng = nc.dram_tensor("ln_g_row", [1, C], F32, kind="ExternalInput")
    lnb = nc.dram_tensor("ln_b_row", [1, C], F32, kind="ExternalInput")
    qkw = nc.dram_tensor("qk_w_tiled", [H, 16, 128, KT, 128], F8, kind="ExternalInput")
    vw = nc.dram_tensor("v_wT", [H, C, C], F8, kind="ExternalInput")
    qb = nc.dram_tensor("q_b_col", [H, 128, 8], F32, kind="ExternalInput")
    kb = nc.dram_tensor("k_b_col", [H, 128, 8], F32, kind="ExternalInput")
    vb = nc.dram_tensor("v_b_row", [H, 1, C], BF16, kind="ExternalInput")
    pw = nc.dram_tensor("proj_w_tiled", [H, 8, 128, KT, 128], F8, kind="ExternalInput")
    pb = nc.dram_tensor("proj_b_col", [H, 128, 8], F32, kind="ExternalInput")
    fow = nc.dram_tensor("fc_out_wT", [C, C], F8, kind="ExternalInput")
    fob = nc.dram_tensor("fc_out_b_row", [1, C], BF16, kind="ExternalInput")
    pe = nc.dram_tensor("pe", [S_OWN, C], F32, kind="ExternalInput")
    out = nc.dram_tensor("out", [S_OWN, C], F32, kind="ExternalOutput")

    def mm2(ps, lhsT, rhs, start, stop):
        nc.tensor.matmul(ps, lhsT, rhs, start=start, stop=stop, perf_mode=DROW)

    with tile.TileContext(nc) as tc:
        with (
            tc.tile_pool(name="dram", bufs=1, space="DRAM") as dram,
            tc.tile_pool(name="pers", bufs=1) as pers,
            tc.tile_pool(name="hown", bufs=2) as hown_pool,
            tc.tile_pool(name="wts", bufs=2) as wts,
            tc.tile_pool(name="psA", bufs=2, space="PSUM") as psA,
            tc.tile_pool(name="psB", bufs=3, space="PSUM") as psB,
            tc.tile_pool(name="small", bufs=2) as small,
            tc.tile_pool(name="tmp", bufs=2) as tmp,
            tc.tile_pool(name="xch", bufs=2) as xch,
        ):
            # per-head pair exchange buffers (RS sum trick), 2 chunks per head
            bounce_in = [[dram.tile([2, C, SH2], F8, name=f"xin{i}_{ch}")
                          for ch in range(2)] for i in range(n_heads)]
            rs_out = [[dram.tile([C, SH2], F8, name=f"xout{i}_{ch}")
                       for ch in range(2)] for i in range(n_heads)]

            ones_bf = pers.tile([1, 128], BF16)
            nc.vector.memset(ones_bf[:], 1.0)
            # 1/SM_SCALE so the softmax reciprocal comes out pre-scaled
            ones_col = pers.tile([128, 1], BF16)
            nc.vector.memset(ones_col[:], 1.0 / SM_SCALE)
            eps_t = pers.tile([128, 1], F32)
            nc.vector.memset(eps_t[:], 1e-5)
            pred = pers.tile([128, KT, S_OWN], F8)

            def launch_rs(idx, h_src, ch):
                """Send [h;h] of s-chunk ch, ReduceScatter-add across the pair.
                rs_out = h_own + h_partner for those s columns."""
                csl = slice(ch * SH2, (ch + 1) * SH2)
                for dup in range(2):
                    nc.sync.dma_start(
                        bounce_in[idx][ch][dup].rearrange("(k p) s -> p k s", p=128),
                        h_src[:, :, csl])
                nc.gpsimd.collective_compute(
                    "ReduceScatter", ADD, replica_groups=PAIRS,
                    ins=[bounce_in[idx][ch][:].opt()], outs=[rs_out[idx][ch][:].opt()],
                )

            def load_weights(j):
                """One big DMA per weight tensor for head j (prefetchable)."""
                wj = j % H
                wqk_sb = wts.tile([128, 16, KT, 128], F8, tag="wqk")
                nc.sync.dma_start(wqk_sb[:], qkw[wj].rearrange("c p k f -> p c k f"))
                wv_sb = wts.tile([128, KT, C], F8, tag="wv")
                nc.sync.dma_start(wv_sb[:], vw[wj].rearrange("(k p) n -> p k n", p=128))
                wp_sb = wts.tile([128, 8, KT, 128], F8, tag="wp")
                nc.sync.dma_start(wp_sb[:], pw[wj].rearrange("c p k f -> p c k f"))
                return wqk_sb, wv_sb, wp_sb

            # -------- stage 0: fc_in + LayerNorm over OWN rows only ------
            h_own = hown_pool.tile([128, KT, S_OWN], F8, tag="hown", name="hT0")
            with tc.tile_pool(name="stage0", bufs=1) as s0:
                x_sb = s0.tile([128, KT, S_OWN], F8)
                nc.sync.dma_start(x_sb[:], x_t[:].rearrange("(k p) s -> p k s", p=128))
                fcw_sb = s0.tile([128, KT, C], F8)
                nc.sync.dma_start(fcw_sb[:], fcw[:].rearrange("(k p) n -> p k n", p=128))
                if with_bias:
                    fcb_sb = s0.tile([1, C], BF16)
                    nc.sync.dma_start(fcb_sb[:], fcb[:])
                if with_ln_affine:
                    g_bc = s0.tile([128, C], F32)
                    nc.sync.dma_start(g_bc[:], bass.AP(tensor=lng, offset=0,
                                                       ap=[[0, 128], [1, C]]))
                    b_bc = s0.tile([128, C], F32)
                    nc.sync.dma_start(b_bc[:], bass.AP(tensor=lnb, offset=0,
                                                       ap=[[0, 128], [1, C]]))
                w_cur = load_weights(0)

                for ss in range(8):
                    ps = psA.tile([128, C], F32, tag="mmA")
                    for kk in range(0, KT, 2):
                        for nch in range(2):
                            nsl = slice(nch * 512, (nch + 1) * 512)
                            mm2(ps[:, nsl], x_sb[:, kk:kk + 2, ss * 128:(ss + 1) * 128],
                                fcw_sb[:, kk:kk + 2, nsl], start=(kk == 0),
                                stop=(not with_bias and kk == KT - 2))
                    if with_bias:
                        for nch in range(2):
                            nsl = slice(nch * 512, (nch + 1) * 512)
                            nc.tensor.matmul(ps[:, nsl], ones_bf[:], fcb_sb[0:1, nsl],
                                             start=False, stop=True)
                    stats = small.tile([128, 2, 6], F32, tag="bnst")
                    nc.vector.bn_stats(stats[:, 0, :], ps[:, 0:512])
                    nc.vector.bn_stats(stats[:, 1, :], ps[:, 512:1024])
                    mv = small.tile([128, 2], F32, tag="mv")
                    nc.vector.bn_aggr(mv[:], stats[:])
                    rstd = small.tile([128, 1], F32, tag="rstd")
                    nc.scalar.activation(rstd[:], mv[:, 1:2], SQRT, bias=eps_t[:], scale=1.0)
                    nc.vector.reciprocal(rstd[:], rstd[:])
                    hnb = s0.tile([128, C], BF16, tag="hnb", bufs=2)
                    if with_ln_affine:
                        hn = s0.tile([128, C], F32, tag="hn", bufs=2)
                        nc.vector.tensor_scalar(hn[:], ps[:], mv[:, 0:1], rstd[:],
                                                op0=SUB, op1=MULT)
                        nc.vector.tensor_mul(hn[:], hn[:], g_bc[:])
                        nc.vector.tensor_add(hnb[:], hn[:], b_bc[:])
                    else:
                        # (x - mu)*rstd on the scalar engine: in*rstd + (-mu*rstd)
                        nmu_rs = small.tile([128, 1], F32, tag="nmurs")
                        nc.vector.tensor_scalar(nmu_rs[:], mv[:, 0:1], rstd[:], -1.0,
                                                op0=MULT, op1=MULT)
                        nc.scalar.activation(hnb[:], ps[:], IDENT,
                                             bias=nmu_rs[:], scale=rstd[:])
                    # bf16 transpose staging (DMA transpose needs 2-byte), then
                    # one DVE pass converts the s-block to fp8
                    h_stg = s0.tile([128, KT, 128], BF16, tag="hstg", bufs=3)
                    nc.sync.dma_start(h_stg[:], hnb[:], transpose=True)
                    nc.vector.tensor_copy(h_own[:, :, ss * 128:(ss + 1) * 128], h_stg[:])
                    if ss == 3:
                        launch_rs(0, h_own, 0)
                launch_rs(0, h_own, 1)

            # ---------------- heads ----------------
            for i in range(n_heads):
                with (
                    tc.tile_pool(name=f"head{i}", bufs=1) as hp,
                    tc.tile_pool(name=f"attT{i}", bufs=1) as attp,
                ):
                    wqk_sb, wv_sb, wp_sb = w_cur
                    if i + 1 < n_heads:
                        w_cur = load_weights(i + 1)
                    q_sb = hp.tile([128, 8, S_OWN], F8, name="q_sb")
                    k_sb = hp.tile([128, 8, T], F8, name="k_sb")
                    v_sb = hp.tile([128, 16, C], F8, name="v_sb")
                    sc_sb = hp.tile([128, 16, S_OWN], F8, name="sc_sb")
                    if with_bias:
                        vb_sb = small.tile([1, C], BF16, tag="vb", bufs=1)
                        nc.sync.dma_start(vb_sb[:], vb[i % H])
                    qb_sb = small.tile([128, 8], F32, tag="qb")
                    nc.sync.dma_start(qb_sb[:], qb[i % H])
                    kb_sb = small.tile([128, 8], F32, tag="kb")
                    nc.sync.dma_start(kb_sb[:], kb[i % H])
                    pb_sb = small.tile([128, 8], F32, tag="pb")
                    nc.sync.dma_start(pb_sb[:], pb[i % H])

                    def kv_block(hh, tloc):
                        """k^T and v for 512 t-rows given their h^T [128,KT,512].
                        tloc: t-tile base (in units of 128 rows) / 4."""
                        tsl = slice(tloc * 512, (tloc + 1) * 512)
                        for co in range(8):
                            ps = psB.tile([128, 512], F32, tag="mmB", name="psk")
                            for kk in range(0, KT, 2):
                                mm2(ps[:], wqk_sb[:, 8 + co, kk:kk + 2, :],
                                    hh[:, kk:kk + 2, :], start=(kk == 0),
                                    stop=(kk == KT - 2))
                            nc.scalar.activation(k_sb[:, co, tsl], ps[:], GELU,
                                                 bias=kb_sb[:, co:co + 1], scale=1.0)
                        for tt in range(4):
                            ps = psA.tile([128, C], F32, tag="mmA")
                            for kk in range(0, KT, 2):
                                for nch in range(2):
                                    nsl = slice(nch * 512, (nch + 1) * 512)
                                    mm2(ps[:, nsl], hh[:, kk:kk + 2, tt * 128:(tt + 1) * 128],
                                        wv_sb[:, kk:kk + 2, nsl], start=(kk == 0),
                                        stop=(not with_bias and kk == KT - 2))
                            if with_bias:
                                for nch in range(2):
                                    nsl = slice(nch * 512, (nch + 1) * 512)
                                    nc.tensor.matmul(ps[:, nsl], ones_bf[:], vb_sb[0:1, nsl],
                                                     start=False, stop=True)
                            nc.scalar.activation(v_sb[:, tloc * 4 + tt, :], ps[:], GELU)

                    def scores(tt_range, sh):
                        """scores^T for t-tiles tt_range into s-half sh."""
                        ssl = slice(sh * 512, (sh + 1) * 512)
                        for tt in tt_range:
                            ps = psB.tile([128, 512], F32, tag="mmB", name="pss")
                            for cc in range(0, 8, 2):
                                mm2(ps[:], k_sb[:, cc:cc + 2, tt * 128:(tt + 1) * 128],
                                    q_sb[:, cc:cc + 2, ssl], start=(cc == 0), stop=(cc == 6))
                            nc.vector.tensor_scalar_mul(sc_sb[:, tt, ssl], ps[:], 1.0 / C)

                    # ---- phase A: own rows only (no collective dependency)
                    for co in range(8):
                        ps = psA.tile([128, C], F32, tag="mmA")
                        for kk in range(0, KT, 2):
                            for nch in range(2):
                                nsl = slice(nch * 512, (nch + 1) * 512)
                                mm2(ps[:, nsl], wqk_sb[:, co, kk:kk + 2, :],
                                    h_own[:, kk:kk + 2, nsl],
                                    start=(kk == 0), stop=(kk == KT - 2))
                        nc.scalar.activation(q_sb[:, co, :], ps[:], GELU,
                                             bias=qb_sb[:, co:co + 1], scale=1.0)
                    for ch in range(2):
                        kv_block(h_own[:, :, ch * 512:(ch + 1) * 512], ch)
                    scores(range(8), 0)
                    scores(range(8), 1)

                    # ---- phase B: partner rows via the RS sum trick
                    for ch in range(2):
                        rsum_sb = xch.tile([128, KT, 512], F8, tag="rsum")
                        nc.sync.dma_start(rsum_sb[:],
                                          rs_out[i][ch][:].rearrange("(k p) s -> p k s", p=128))
                        hp_sb = xch.tile([128, KT, 512], F8, tag="hpart")
                        nc.gpsimd.tensor_sub(hp_sb[:], rsum_sb[:],
                                             h_own[:, :, ch * SH2:(ch + 1) * SH2])
                        kv_block(hp_sb, 2 + ch)
                    scores(range(8, 16), 0)
                    scores(range(8, 16), 1)

                    # ---- att^T, channel softmax, proj per s-half
                    h_new = hown_pool.tile([128, KT, S_OWN], F8, tag="hown", name=f"hT{i + 1}")
                    attsmT = attp.tile([128, KT, S_OWN], BF16, tag="attT", name="attsmT")
                    attn8 = attp.tile([128, KT, S_OWN], F8, tag="attn8", name="attn8")

                    for sh in range(2):
                        ssl = slice(sh * 512, (sh + 1) * 512)
                        for co in range(8):
                            ps = psB.tile([128, 512], F32, tag="mmB", name="psa")
                            for tt in range(0, 16, 2):
                                mm2(ps[:], v_sb[:, tt:tt + 2, co * 128:(co + 1) * 128],
                                    sc_sb[:, tt:tt + 2, ssl],
                                    start=(tt == 0), stop=(tt == 14))
                            nc.scalar.activation(attsmT[:, co, ssl], ps[:], EXP)
                        dn = psB.tile([1, 512], F32, tag="denom", bufs=1, name="dn")
                        for co in range(8):
                            nc.tensor.matmul(dn[:], ones_col[:], attsmT[:, co, ssl],
                                             start=(co == 0), stop=(co == 7))
                        rr = small.tile([1, 512], F32, tag="rr", name="rr")
                        nc.vector.reciprocal(rr[:], dn[:])
                        rrb = small.tile([1, 512], BF16, tag="rrb", name="rrb")
                        nc.vector.tensor_copy(rrb[:], rr[:])
                        bc = psB.tile([128, 512], F32, tag="mmB", name="bc")
                        nc.tensor.matmul(bc[:], ones_bf[:], rrb[:], start=True, stop=True)
                        for co in range(8):
                            nc.vector.tensor_mul(attn8[:, co, ssl], attsmT[:, co, ssl], bc[:])

                        for co in range(8):
                            ps = psB.tile([128, 512], F32, tag="mmB", name="psp")
                            for cc in range(0, 8, 2):
                                mm2(ps[:], wp_sb[:, co, cc:cc + 2, :],
                                    attn8[:, cc:cc + 2, ssl], start=(cc == 0), stop=(cc == 6))
                            if i == 0:
                                nc.scalar.activation(h_new[:, co, ssl], ps[:], GELU,
                                                     bias=pb_sb[:, co:co + 1],
                                                     scale=1.0 / SM_SCALE)
                                nc.vector.tensor_copy(pred[:, co, ssl], h_new[:, co, ssl])
                            else:
                                gtmp = tmp.tile([128, 512], BF16, tag="gtmp", name="gtmp")
                                nc.scalar.activation(gtmp[:], ps[:], GELU,
                                                     bias=pb_sb[:, co:co + 1],
                                                     scale=1.0 / SM_SCALE)
                                nc.vector.tensor_add(h_new[:, co, ssl], gtmp[:],
                                                     pred[:, co, ssl])
                        if i + 1 < n_heads:
                            launch_rs(i + 1, h_new, sh)
                    h_own = h_new

            # ---------------- fc_out + pose encoding ----------------
            with (
                tc.tile_pool(name="fco", bufs=1) as fo,
                tc.tile_pool(name="fco2", bufs=2) as fo2,
            ):
                fow_sb = fo.tile([128, KT, C], F8)
                nc.sync.dma_start(fow_sb[:], fow[:].rearrange("(k p) n -> p k n", p=128))
                pe_sb = fo.tile([128, 8, C], F32)
                nc.sync.dma_start(pe_sb[:], pe[:].rearrange("(s p) c -> p s c", p=128))
                if with_bias:
                    fob_sb = fo.tile([1, C], BF16)
                    nc.sync.dma_start(fob_sb[:], fob[:])
                for ss in range(8):
                    ps = psA.tile([128, C], F32, tag="mmA")
                    for kk in range(0, KT, 2):
                        for nch in range(2):
                            nsl = slice(nch * 512, (nch + 1) * 512)
                            mm2(ps[:, nsl], h_own[:, kk:kk + 2, ss * 128:(ss + 1) * 128],
                                fow_sb[:, kk:kk + 2, nsl], start=(kk == 0),
                                stop=(not with_bias and kk == KT - 2))
                    if with_bias:
                        for nch in range(2):
                            nsl = slice(nch * 512, (nch + 1) * 512)
                            nc.tensor.matmul(ps[:, nsl], ones_bf[:], fob_sb[0:1, nsl],
                                             start=False, stop=True)
                    o_sb = fo2.tile([128, C], F32, tag="osb")
                    nc.vector.tensor_add(o_sb[:], ps[:], pe_sb[:, ss, :])
                    nc.sync.dma_start(out[ss * 128:(ss + 1) * 128, :], o_sb[:])

    nc.compile()
    return nc


def build_null() -> bacc.Bacc:
    """Same I/O signature, ~no compute: measures the dispatch floor."""
    nc = bacc.Bacc(num_devices=N_CORES, name="attn_null")
    nc.dram_tensor("x_t", [C, S_OWN], F8, kind="ExternalInput")
    nc.dram_tensor("fc_in_wT", [C, C], F8, kind="ExternalInput")
    nc.dram_tensor("fc_in_b_row", [1, C], BF16, kind="ExternalInput")
    nc.dram_tensor("ln_g_row", [1, C], F32, kind="ExternalInput")
    nc.dram_tensor("ln_b_row", [1, C], F32, kind="ExternalInput")
    nc.dram_tensor("qk_w_tiled", [H, 16, 128, KT, 128], F8, kind="ExternalInput")
    nc.dram_tensor("v_wT", [H, C, C], F8, kind="ExternalInput")
    nc.dram_tensor("q_b_col", [H, 128, 8], F32, kind="ExternalInput")
    nc.dram_tensor("k_b_col", [H, 128, 8], F32, kind="ExternalInput")
    nc.dram_tensor("v_b_row", [H, 1, C], BF16, kind="ExternalInput")
    nc.dram_tensor("proj_w_tiled", [H, 8, 128, KT, 128], F8, kind="ExternalInput")
    nc.dram_tensor("proj_b_col", [H, 128, 8], F32, kind="ExternalInput")
    nc.dram_tensor("fc_out_wT", [C, C], F8, kind="ExternalInput")
    nc.dram_tensor("fc_out_b_row", [1, C], BF16, kind="ExternalInput")
    pe = nc.dram_tensor("pe", [S_OWN, C], F32, kind="ExternalInput")
    out = nc.dram_tensor("out", [S_OWN, C], F32, kind="ExternalOutput")
    with tile.TileContext(nc) as tc:
        with tc.tile_pool(name="p", bufs=2) as p:
            for ss in range(8):
                t = p.tile([128, C], F32, tag="t")
                nc.sync.dma_start(t[:], pe[ss * 128:(ss + 1) * 128, :])
                nc.sync.dma_start(out[ss * 128:(ss + 1) * 128, :], t[:])
    nc.compile()
    return nc


def _pose_enc_np(s, f):
    pos = np.arange(s, dtype=np.float32)[:, None]
    div = (1.0 / (1000.0 ** (2.0 * np.arange(f, dtype=np.float32) / np.float32(f))))[None, :]
    p = np.zeros((s, f), np.float32)
    p[0::2, :] = np.sin(pos[0::2] * div)
    p[1::2, :] = np.cos(pos[1::2] * div)
    return p


def _bf(a):
    return np.ascontiguousarray(np.asarray(a, np.float32).astype(NP_BF16))


def _f8(a):
    return np.ascontiguousarray(np.asarray(a, np.float32).astype(NP_F8))


def _f32(a):
    return np.ascontiguousarray(np.asarray(a, np.float32))


def prepare_in_maps(x, fc_in_w, fc_in_b, ln_g, ln_b, qkv_w, qkv_b, proj_w, proj_b,
                    fc_out_w, fc_out_b):
    x = np.asarray(x, np.float32)
    qkv_w = np.asarray(qkv_w, np.float32)
    qkv_b = np.asarray(qkv_b, np.float32)
    proj_w = np.asarray(proj_w, np.float32)

    # [H, c_in, 2C] with q columns then k columns -> [H, 16, 128, KT, 128]
    qkT = np.concatenate([qkv_w[:, 0:C, :].transpose(0, 2, 1),
                          qkv_w[:, C:2 * C, :].transpose(0, 2, 1)], axis=2)
    qk_tiled = _f8(qkT.reshape(H, KT, 128, 16, 128).transpose(0, 3, 2, 1, 4))
    v_wT = _f8(qkv_w[:, 2 * C:, :].transpose(0, 2, 1))
    pw_tiled = _f8(proj_w.transpose(0, 2, 1).reshape(H, KT, 128, 8, 128).transpose(0, 3, 2, 1, 4))

    shared = {
        "fc_in_wT": _f8(np.asarray(fc_in_w, np.float32).T),
        "fc_in_b_row": _bf(np.asarray(fc_in_b)[None, :]),
        "ln_g_row": _f32(np.asarray(ln_g)[None, :]),
        "ln_b_row": _f32(np.asarray(ln_b)[None, :]),
        "qk_w_tiled": qk_tiled,
        "v_wT": v_wT,
        "q_b_col": _f32(qkv_b[:, 0:C].reshape(H, 8, 128).transpose(0, 2, 1)),
        "k_b_col": _f32(qkv_b[:, C:2 * C].reshape(H, 8, 128).transpose(0, 2, 1)),
        "v_b_row": _bf(qkv_b[:, 2 * C:][:, None, :]),
        "proj_w_tiled": pw_tiled,
        "proj_b_col": _f32(np.asarray(proj_b, np.float32).reshape(H, 8, 128).transpose(0, 2, 1)),
        "fc_out_wT": _f8(np.asarray(fc_out_w, np.float32).T),
        "fc_out_b_row": _bf(np.asarray(fc_out_b)[None, :]),
    }
    pe_full = _pose_enc_np(S, C)
    in_maps = []
    for core in range(N_CORES):
        b, half = divmod(core, 2)
        m = dict(shared)
        m["x_t"] = _f8(x[b, half * S_OWN:(half + 1) * S_OWN, :].T)
        m["pe"] = np.ascontiguousarray(pe_full[half * S_OWN:(half + 1) * S_OWN, :])
        in_maps.append(m)
    return in_maps


_NC_CACHE = {}


def get_nc(n_heads=H, with_bias=True, with_ln_affine=True):
    key = (n_heads, with_bias, with_ln_affine)
    if key not in _NC_CACHE:
        _NC_CACHE[key] = build(n_heads, with_bias, with_ln_affine)
    return _NC_CACHE[key]


_EXEC_CACHE = {}


def _get_executable(nc):
    """One jitted collectives executable per process (loading a second one
    hangs the axon worker); reused across kernel() calls."""
    key = id(nc)
    if key in _EXEC_CACHE:
        return _EXEC_CACHE[key]
    import jax
    from jax.sharding import Mesh, PartitionSpec, NamedSharding
    from jax.experimental.shard_map import shard_map
    from concourse import bass2jax
    import concourse.mybir as mybir_

    bass2jax.install_neuronx_cc_hook()
    partition_name = nc.partition_id_tensor.name if nc.partition_id_tensor else None
    in_names, out_names, out_avals, zero_outs = [], [], [], []
    for alloc in nc.m.functions[0].allocations:
        if not isinstance(alloc, mybir_.MemoryLocationSet):
            continue
        name = alloc.memorylocations[0].name
        if alloc.kind == "ExternalInput":
            if name != partition_name:
                in_names.append(name)
        elif alloc.kind == "ExternalOutput":
            out_names.append(name)
            shape = tuple(alloc.tensor_shape)
            dtype = mybir_.dt.np(alloc.dtype)
            out_avals.append(jax.core.ShapedArray(shape, dtype))
            zero_outs.append(np.zeros(shape, dtype))
    n_params = len(in_names)
    n_outs = len(out_avals)
    all_in = in_names + out_names + ([partition_name] if partition_name else [])
    donate = tuple(range(n_params, n_params + n_outs))

    def _body(*args):
        operands = list(args)
        if partition_name is not None:
            operands.append(bass2jax.partition_id_tensor())
        return tuple(bass2jax._bass_exec_p.bind(
            *operands, out_avals=tuple(out_avals), in_names=tuple(all_in),
            out_names=tuple(out_names), lowering_input_output_aliases=(),
            sim_require_finite=True, sim_require_nnan=True, nc=nc))

    devices = jax.devices()[:N_CORES]
    mesh = Mesh(np.asarray(devices), ("core",))
    sharded = jax.jit(
        shard_map(_body, mesh=mesh,
                  in_specs=(PartitionSpec("core"),) * (n_params + n_outs),
                  out_specs=(PartitionSpec("core"),) * len(out_names),
                  check_rep=False),
        donate_argnums=donate, keep_unused=True)
    sh = NamedSharding(mesh, PartitionSpec("core"))
    entry = (sharded, sh, in_names[:n_params], out_names, out_avals, zero_outs)
    _EXEC_CACHE[key] = entry
    return entry


def flags_for(inputs):
    with_bias = not (np.all(np.asarray(inputs["fc_in_b"]) == 0)
                     and np.all(np.asarray(inputs["qkv_b"]) == 0)
                     and np.all(np.asarray(inputs["proj_b"]) == 0)
                     and np.all(np.asarray(inputs["fc_out_b"]) == 0))
    with_ln = not (np.all(np.asarray(inputs["ln_g"]) == 1)
                   and np.all(np.asarray(inputs["ln_b"]) == 0))
    return with_bias, with_ln


def kernel(**inputs) -> np.ndarray:
    with_bias, with_ln = flags_for(inputs)
    nc = get_nc(H, with_bias, with_ln)
    in_maps = prepare_in_maps(**inputs)
    import jax
    sharded, sh, in_names, out_names, out_avals, zero_outs = _get_executable(nc)
    concat_in = [jax.device_put(
        np.concatenate([np.asarray(in_maps[c][nm]) for c in range(N_CORES)], axis=0), sh)
        for nm in in_names]
    concat_zeros = [jax.device_put(
        np.zeros((N_CORES * z.shape[0], *z.shape[1:]), z.dtype), sh) for z in zero_outs]
    out_arrs = sharded(*concat_in, *concat_zeros)
    jax.block_until_ready(out_arrs)
    oi = out_names.index("out")
    per_core = np.asarray(out_arrs[oi]).reshape(N_CORES, *out_avals[oi].shape)
    out_full = np.empty((B, S, C), np.float32)
    for core in range(N_CORES):
        b, half = divmod(core, 2)
        out_full[b, half * S_OWN:(half + 1) * S_OWN, :] = per_core[core]
    return out_full


# revision 41
# speedup vs baseline: 15.4827x; 12.4453x over previous
"""Distributed Trainium2 kernel for nn_Attention_18562848653411.

Reference model: fc_in -> LayerNorm -> 4 sequential "refinement heads"
(qkv matmul + gelu, scores=q@k^T/C, att=scores@v, softmax over channels,
proj + gelu, residual with head-0 output) -> fc_out + PoseEncoding.

Sharding (8 NeuronCores): core c handles batch b=c//2, sequence half
h=c%2 (1024 of 2048 rows). All weights replicated; every stage is
row-local except k/v, which need h for the full sequence.

Pair exchange runs as a ReduceScatter sum-trick instead of AllGather:
each core sends its h^T chunk DUPLICATED ([h; h], so the collective is
rank-symmetric), receives sum = h_own + h_partner, and recovers the
partner half with subtracts split across DVE and GpSimd. This keeps
the graph SPMD-uniform with the partner data at a fixed t-slot, moves
half the collective bytes of an AllGather, and lets all own-row work
run before the collective lands: q, own-row k/v, own-t scores, AND the
own-t half of the att accumulation, which is staged to fp8 and folded
back into the partner-half PSUM group with an identity matmul.
fc_in+LayerNorm is computed for own rows only (its own 4-deep PSUM
pool, freed before the head pools open) and head 0 consumes the same
exchange as every other head. Per-head weights arrive as three large
prefetched DMAs; elementwise work is spread across DVE/GpSimd/Scalar
(the softmax normalize and residual adds are split between DVE and
GpSimd); gelu/exp activations process merged [128,1024] co-pairs and
all exp calls run back-to-back so each head pays only two activation
table loads.

Activations are kept in transposed [C, S] layout so every matmul
consumes operands natively (contraction on partitions); att^T is
produced directly with v-stationary matmuls and the channel softmax
uses a ones-matmul denominator + rank-1 broadcast (att <= ~10, so exp
needs no max subtraction).

Compute: fp8e4 (e4m3) matmuls with DoubleRow perf mode (2 fp8 weights
per PE cell -> 2 k-tiles per instruction) and f32 PSUM accumulation.
The channel softmax renormalizes away quantization noise each head and
the exact-f32 pose encoding dominates the output, so end-to-end rel
err stays ~3.5e-4 (measured vs the f32 reference; gate is 2e-2). The
softmax probabilities are stored scaled by 64 so they sit in fp8e4's
normal range; the proj activation applies the compensating 1/64 via
its input scale. exp() values (up to e^10) stay bf16. DMAs are batched
into few large multi-dim transfers (the DGE costs ~625ns per
instruction regardless of size).
"""

import numpy as np
import ml_dtypes

import concourse.bass as bass
import concourse.mybir as mybir
import concourse.tile as tile
from concourse import bacc
from concourse.bass_utils import run_bass_kernel_spmd

N_CORES = 8
PAIRS = [[0, 1], [2, 3], [4, 5], [6, 7]]
B, S, C = 4, 2048, 1024
H = 4
S_OWN = S // 2        # rows per core
T = S                 # full sequence (k/v length)
KT = C // 128         # contraction tiles
SH2 = S_OWN // 2      # exchange chunk (s columns)

F32 = mybir.dt.float32
BF16 = mybir.dt.bfloat16
F8 = mybir.dt.float8e4
GELU = mybir.ActivationFunctionType.Gelu
EXP = mybir.ActivationFunctionType.Exp
SQRT = mybir.ActivationFunctionType.Sqrt
IDENT = mybir.ActivationFunctionType.Identity
SUB = mybir.AluOpType.subtract
MULT = mybir.AluOpType.mult
ADD = mybir.AluOpType.add
BYPASS = mybir.AluOpType.bypass
DROW = mybir.MatmulPerfMode.DoubleRow

NP_BF16 = ml_dtypes.bfloat16
NP_F8 = ml_dtypes.float8_e4m3

SM_SCALE = 64.0       # softmax store scale (fp8e4 normal range)

import os
V_NORM_ACT = os.environ.get("V_NORM_ACT", "1") == "1"   # LN normalize on scalar engine
V_TRANS_SP = os.environ.get("V_TRANS_SP", "1") == "1"   # transposes on SP queue
V_SUB_JOINT = os.environ.get("V_SUB_JOINT", "0") == "1" # single Pool subtract
V_H0_OLD = os.environ.get("V_H0_OLD", "1") == "1"       # head0 phase order like other heads


def build(n_heads: int = H, with_bias: bool = True, with_ln_affine: bool = True) -> bacc.Bacc:
    """Build the SPMD graph. n_heads>4 cycles weights (timing builds).
    with_bias=False elides the K=1 bias matmuls (all harness biases are 0);
    with_ln_affine=False elides the LayerNorm gain/bias application."""
    nc = bacc.Bacc(num_devices=N_CORES, name="attn")

    x_t = nc.dram_tensor("x_t", [C, S_OWN], F8, kind="ExternalInput")
    fcw = nc.dram_tensor("fc_in_wT", [C, C], F8, kind="ExternalInput")
    fcb = nc.dram_tensor("fc_in_b_row", [1, C], BF16, kind="ExternalInput")
    lng = nc.dram_tensor("ln_g_row", [1, C], F32, kind="ExternalInput")
    lnb = nc.dram_tensor("ln_b_row", [1, C], F32, kind="ExternalInput")
    qkw = nc.dram_tensor("qk_w_tiled", [H, 16, 128, KT, 128], F8, kind="ExternalInput")
    vw = nc.dram_tensor("v_wT", [H, C, C], F8, kind="ExternalInput")
    qb = nc.dram_tensor("q_b_col", [H, 128, 8], F32, kind="ExternalInput")
    kb = nc.dram_tensor("k_b_col", [H, 128, 8], F32, kind="ExternalInput")
    vb = nc.dram_tensor("v_b_row", [H, 1, C], BF16, kind="ExternalInput")
    pw = nc.dram_tensor("proj_w_tiled", [H, 8, 128, KT, 128], F8, kind="ExternalInput")
    pb = nc.dram_tensor("proj_b_col", [H, 128, 8], F32, kind="ExternalInput")
    fow = nc.dram_tensor("fc_out_wT", [C, C], F8, kind="ExternalInput")
    fob = nc.dram_tensor("fc_out_b_row", [1, C], BF16, kind="ExternalInput")
    pe = nc.dram_tensor("pe", [S_OWN, C], F32, kind="ExternalInput")
    ident = nc.dram_tensor("ident128", [128, 128], F8, kind="ExternalInput")
    out = nc.dram_tensor("out", [S_OWN, C], F32, kind="ExternalOutput")

    def mm2(ps, lhsT, rhs, start, stop):
        nc.tensor.matmul(ps, lhsT, rhs, start=start, stop=stop, perf_mode=DROW)

    with tile.TileContext(nc) as tc:
        with (
            tc.tile_pool(name="dram", bufs=1, space="DRAM") as dram,
            tc.tile_pool(name="pers", bufs=1) as pers,
            tc.tile_pool(name="hown", bufs=2) as hown_pool,
            tc.tile_pool(name="wts", bufs=2) as wts,
            tc.tile_pool(name="small", bufs=2) as small,
            tc.tile_pool(name="tmp", bufs=2) as tmp,
            tc.tile_pool(name="xch", bufs=1 if os.environ.get("V_XCH1", "0") == "1" else 2) as xch,
        ):
            # per-head pair exchange buffers (RS sum trick)
            XCH1 = os.environ.get("V_XCH1", "0") == "1"
            CW = S_OWN if XCH1 else SH2  # exchange chunk width (s columns)
            NCH = 1 if XCH1 else 2
            bounce_in = [[dram.tile([2, C, CW], F8, name=f"xin{i}_{ch}")
                          for ch in range(NCH)] for i in range(n_heads)]
            rs_out = [[dram.tile([C, CW], F8, name=f"xout{i}_{ch}")
                       for ch in range(NCH)] for i in range(n_heads)]

            ones_bf = pers.tile([1, 128], BF16)
            nc.vector.memset(ones_bf[:], 1.0)
            # 1/SM_SCALE so the softmax reciprocal comes out pre-scaled
            ones_col = pers.tile([128, 1], BF16)
            nc.vector.memset(ones_col[:], 1.0 / SM_SCALE)
            eps_t = pers.tile([128, 1], F32)
            nc.vector.memset(eps_t[:], 1e-5)
            pred = pers.tile([128, KT, S_OWN], F8)
            ident_sb = pers.tile([128, 128], F8)
            nc.sync.dma_start(ident_sb[:], ident[:])

            def launch_rs(idx, h_src, ch):
                """Send [h;h] of s-chunk ch, ReduceScatter-add across the pair.
                rs_out = h_own + h_partner for those s columns."""
                csl = slice(ch * CW, (ch + 1) * CW)
                beng = nc.sync if os.environ.get("V_BOUNCE_SP", "1") == "1" else nc.gpsimd
                for dup in range(2):
                    beng.dma_start(
                        bounce_in[idx][ch][dup].rearrange("(k p) s -> p k s", p=128),
                        h_src[:, :, csl])
                nc.gpsimd.collective_compute(
                    "ReduceScatter", ADD, replica_groups=PAIRS,
                    ins=[bounce_in[idx][ch][:].opt()], outs=[rs_out[idx][ch][:].opt()],
                )

            def load_weights_qk(j):
                wqk_sb = wts.tile([128, 16, KT, 128], F8, tag="wqk")
                nc.sync.dma_start(wqk_sb[:], qkw[j % H].rearrange("c p k f -> p c k f"))
                return wqk_sb

            def load_weights_vp(j):
                wj = j % H
                wv_sb = wts.tile([128, KT, C], F8, tag="wv")
                nc.sync.dma_start(wv_sb[:], vw[wj].rearrange("(k p) n -> p k n", p=128))
                wp_sb = wts.tile([128, 8, KT, 128], F8, tag="wp")
                nc.sync.dma_start(wp_sb[:], pw[wj].rearrange("c p k f -> p c k f"))
                return wv_sb, wp_sb

            def load_weights(j):
                wqk_sb = load_weights_qk(j)
                wv_sb, wp_sb = load_weights_vp(j)
                return wqk_sb, wv_sb, wp_sb

            # -------- stage 0: fc_in + LayerNorm over OWN rows only ------
            h_own = hown_pool.tile([128, KT, S_OWN], F8, tag="hown", name="hT0")
            ps0_cm = tc.tile_pool(name="ps0", bufs=4, space="PSUM")
            ps0 = ps0_cm.__enter__()
            with tc.tile_pool(name="stage0", bufs=1) as s0:
                x_sb = s0.tile([128, KT, S_OWN], F8)
                nc.sync.dma_start(x_sb[:, :, 0:512],
                                  x_t[:, 0:512].rearrange("(k p) s -> p k s", p=128))
                fcw_sb = s0.tile([128, KT, C], F8)
                nc.sync.dma_start(fcw_sb[:], fcw[:].rearrange("(k p) n -> p k n", p=128))
                nc.sync.dma_start(x_sb[:, :, 512:1024],
                                  x_t[:, 512:1024].rearrange("(k p) s -> p k s", p=128))
                if with_bias:
                    fcb_sb = s0.tile([1, C], BF16)
                    nc.sync.dma_start(fcb_sb[:], fcb[:])
                if with_ln_affine:
                    g_bc = s0.tile([128, C], F32)
                    nc.sync.dma_start(g_bc[:], bass.AP(tensor=lng, offset=0,
                                                       ap=[[0, 128], [1, C]]))
                    b_bc = s0.tile([128, C], F32)
                    nc.sync.dma_start(b_bc[:], bass.AP(tensor=lnb, offset=0,
                                                       ap=[[0, 128], [1, C]]))
                w_cur = load_weights(0)

                for ss in range(8):
                    ps = ps0.tile([128, C], F32, tag="mm0")
                    for kk in range(0, KT, 2):
                        for nch in range(2):
                            nsl = slice(nch * 512, (nch + 1) * 512)
                            mm2(ps[:, nsl], x_sb[:, kk:kk + 2, ss * 128:(ss + 1) * 128],
                                fcw_sb[:, kk:kk + 2, nsl], start=(kk == 0),
                                stop=(not with_bias and kk == KT - 2))
                    if with_bias:
                        for nch in range(2):
                            nsl = slice(nch * 512, (nch + 1) * 512)
                            nc.tensor.matmul(ps[:, nsl], ones_bf[:], fcb_sb[0:1, nsl],
                                             start=False, stop=True)
                    stats = small.tile([128, 2, 6], F32, tag="bnst")
                    nc.vector.bn_stats(stats[:, 0, :], ps[:, 0:512])
                    nc.vector.bn_stats(stats[:, 1, :], ps[:, 512:1024])
                    mv = small.tile([128, 2], F32, tag="mv")
                    nc.vector.bn_aggr(mv[:], stats[:])
                    rstd = small.tile([128, 1], F32, tag="rstd")
                    nc.scalar.activation(rstd[:], mv[:, 1:2], SQRT, bias=eps_t[:], scale=1.0)
                    nc.vector.reciprocal(rstd[:], rstd[:])
                    hnb = s0.tile([128, C], BF16, tag="hnb", bufs=3)
                    if with_ln_affine:
                        hn = s0.tile([128, C], F32, tag="hn", bufs=2)
                        nc.vector.tensor_scalar(hn[:], ps[:], mv[:, 0:1], rstd[:],
                                                op0=SUB, op1=MULT)
                        nc.vector.tensor_mul(hn[:], hn[:], g_bc[:])
                        nc.vector.tensor_add(hnb[:], hn[:], b_bc[:])
                    elif V_NORM_ACT:
                        # (x - mu)*rstd on the scalar engine: in*rstd + (-mu*rstd)
                        nmu_rs = small.tile([128, 1], F32, tag="nmurs")
                        nc.vector.tensor_scalar(nmu_rs[:], mv[:, 0:1], rstd[:], -1.0,
                                                op0=MULT, op1=MULT)
                        nc.scalar.activation(hnb[:], ps[:], IDENT,
                                             bias=nmu_rs[:], scale=rstd[:])
                    else:
                        # (x - mu)*rstd on the vector engine
                        nc.vector.tensor_scalar(hnb[:], ps[:], mv[:, 0:1], rstd[:],
                                                op0=SUB, op1=MULT)
                    # bf16 transpose staging (DMA transpose needs 2-byte), then
                    # one DVE pass converts the s-block to fp8. Issued from the
                    # Activation DGE queue so stage0's exchange DMAs (SP queue)
                    # don't queue behind transpose dependencies.
                    h_stg = s0.tile([128, KT, 128], BF16, tag="hstg", bufs=4)
                    (nc.sync if V_TRANS_SP else nc.scalar).dma_start(
                        h_stg[:], hnb[:], transpose=True)
                    nc.vector.tensor_copy(h_own[:, :, ss * 128:(ss + 1) * 128], h_stg[:])
                    if ss == 3 and not XCH1:
                        launch_rs(0, h_own, 0)
                if XCH1:
                    launch_rs(0, h_own, 0)
                else:
                    launch_rs(0, h_own, 1)
            ps0_cm.__exit__(None, None, None)
            psA_cm = tc.tile_pool(name="psA", bufs=2, space="PSUM")
            psA = psA_cm.__enter__()
            psB_cm = tc.tile_pool(name="psB", bufs=3, space="PSUM")
            psB = psB_cm.__enter__()

            # ---------------- heads ----------------
            for i in range(n_heads):
                with (
                    tc.tile_pool(name=f"head{i}", bufs=1) as hp,
                    tc.tile_pool(name=f"attT{i}", bufs=1) as attp,
                ):
                    wqk_sb, wv_sb, wp_sb = w_cur
                    if i + 1 < n_heads:
                        w_cur = load_weights(i + 1)
                    q_sb = hp.tile([128, 8, S_OWN], F8, name="q_sb")
                    k_sb = hp.tile([128, 8, T], F8, name="k_sb")
                    v_sb = hp.tile([128, 16, C], F8, name="v_sb")
                    sc_sb = hp.tile([128, 16, S_OWN], F8, name="sc_sb")
                    if with_bias:
                        vb_sb = small.tile([1, C], BF16, tag="vb", bufs=1)
                        nc.sync.dma_start(vb_sb[:], vb[i % H])
                    qb_sb = small.tile([128, 8], F32, tag="qb")
                    nc.sync.dma_start(qb_sb[:], qb[i % H])
                    kb_sb = small.tile([128, 8], F32, tag="kb")
                    nc.sync.dma_start(kb_sb[:], kb[i % H])
                    pb_sb = small.tile([128, 8], F32, tag="pb")
                    nc.sync.dma_start(pb_sb[:], pb[i % H])

                    def kv_block(hh, tloc):
                        """k^T and v for 512 t-rows given their h^T [128,KT,512].
                        tloc: t-tile base (in units of 128 rows) / 4."""
                        tsl = slice(tloc * 512, (tloc + 1) * 512)
                        if with_bias:
                            for co in range(8):
                                ps = psB.tile([128, 512], F32, tag="mmB", name="psk")
                                for kk in range(0, KT, 2):
                                    mm2(ps[:], wqk_sb[:, 8 + co, kk:kk + 2, :],
                                        hh[:, kk:kk + 2, :], start=(kk == 0),
                                        stop=(kk == KT - 2))
                                nc.scalar.activation(k_sb[:, co, tsl], ps[:], GELU,
                                                     bias=kb_sb[:, co:co + 1], scale=1.0)
                        else:
                            # co-pairs share one [128,1024] PSUM + one act call
                            for cop in range(0, 8, 2):
                                ps = psA.tile([128, C], F32, tag="mmA", name="psk")
                                for j in range(2):
                                    jsl = slice(j * 512, (j + 1) * 512)
                                    for kk in range(0, KT, 2):
                                        mm2(ps[:, jsl], wqk_sb[:, 8 + cop + j, kk:kk + 2, :],
                                            hh[:, kk:kk + 2, :], start=(kk == 0),
                                            stop=(kk == KT - 2))
                                nc.scalar.activation(k_sb[:, cop:cop + 2, tsl], ps[:], GELU)
                        for tt in range(4):
                            ps = psA.tile([128, C], F32, tag="mmA")
                            for kk in range(0, KT, 2):
                                for nch in range(2):
                                    nsl = slice(nch * 512, (nch + 1) * 512)
                                    mm2(ps[:, nsl], hh[:, kk:kk + 2, tt * 128:(tt + 1) * 128],
                                        wv_sb[:, kk:kk + 2, nsl], start=(kk == 0),
                                        stop=(not with_bias and kk == KT - 2))
                            if with_bias:
                                for nch in range(2):
                                    nsl = slice(nch * 512, (nch + 1) * 512)
                                    nc.tensor.matmul(ps[:, nsl], ones_bf[:], vb_sb[0:1, nsl],
                                                     start=False, stop=True)
                            nc.scalar.activation(v_sb[:, tloc * 4 + tt, :], ps[:], GELU)

                    def scores(tt_range, sh):
                        """scores^T for t-tiles tt_range into s-half sh."""
                        ssl = slice(sh * 512, (sh + 1) * 512)
                        for tt in tt_range:
                            ps = psB.tile([128, 512], F32, tag="mmB", name="pss")
                            for cc in range(0, 8, 2):
                                mm2(ps[:], k_sb[:, cc:cc + 2, tt * 128:(tt + 1) * 128],
                                    q_sb[:, cc:cc + 2, ssl], start=(cc == 0), stop=(cc == 6))
                            nc.vector.tensor_scalar_mul(sc_sb[:, tt, ssl], ps[:], 1.0 / C)

                    def partner_kv(ch):
                        """Recover partner h for chunk ch from the RS sum and
                        run its k/v. DMA + subtract are split in kk-halves
                        across DVE and GpSimd to shorten the critical tail."""
                        rsum_sb = xch.tile([128, KT, CW], F8, tag="rsum")
                        hp_sb = xch.tile([128, KT, CW], F8, tag="hpart")
                        csl = slice(ch * CW, (ch + 1) * CW)
                        half = KT // 2
                        nc.scalar.dma_start(
                            rsum_sb[:, 0:half, :],
                            rs_out[i][ch][0:half * 128].rearrange("(k p) s -> p k s", p=128))
                        nc.sync.dma_start(
                            rsum_sb[:, half:KT, :],
                            rs_out[i][ch][half * 128:C].rearrange("(k p) s -> p k s", p=128))
                        nc.vector.tensor_sub(hp_sb[:, 0:half, :], rsum_sb[:, 0:half, :],
                                             h_own[:, 0:half, csl])
                        nc.gpsimd.tensor_sub(hp_sb[:, half:KT, :], rsum_sb[:, half:KT, :],
                                             h_own[:, half:KT, csl])
                        if XCH1:
                            kv_block(hp_sb[:, :, 0:512], 2)
                            kv_block(hp_sb[:, :, 512:1024], 3)
                        else:
                            kv_block(hp_sb, 2 + ch)

                    h_new = hown_pool.tile([128, KT, S_OWN], F8, tag="hown", name=f"hT{i + 1}")
                    attsmT = attp.tile([128, KT, S_OWN], BF16, tag="attT", name="attsmT")
                    attn8 = attp.tile([128, KT, S_OWN], F8, tag="attn8", name="attn8")
                    att_own8 = attp.tile([128, KT, S_OWN], F8, tag="attown", name="att_own8")

                    def att_own_block():
                        # partial att over OWN t-tiles (0..7): all inputs local,
                        # runs while the pair exchange is still in flight
                        for sh in range(2):
                            ssl = slice(sh * 512, (sh + 1) * 512)
                            for cop in range(0, 8, 2):
                                ps = psA.tile([128, C], F32, tag="mmA", name="psao")
                                for j in range(2):
                                    jsl = slice(j * 512, (j + 1) * 512)
                                    for tt in range(0, 8, 2):
                                        mm2(ps[:, jsl],
                                            v_sb[:, tt:tt + 2, (cop + j) * 128:(cop + j + 1) * 128],
                                            sc_sb[:, tt:tt + 2, ssl],
                                            start=(tt == 0), stop=(tt == 6))
                                nc.vector.tensor_copy(att_own8[:, cop:cop + 2, ssl], ps[:])

                    # ---- per-head schedule: own-row work (q, k/v, own-t
                    # scores) carries the PE while the pair exchange lands;
                    # partner chunk 0 arrives well before chunk 1, so its k/v
                    # run between them.
                    def q_block():
                        if with_bias:
                            for co in range(8):
                                ps = psA.tile([128, C], F32, tag="mmA")
                                for kk in range(0, KT, 2):
                                    for nch in range(2):
                                        nsl = slice(nch * 512, (nch + 1) * 512)
                                        mm2(ps[:, nsl], wqk_sb[:, co, kk:kk + 2, :],
                                            h_own[:, kk:kk + 2, nsl],
                                            start=(kk == 0), stop=(kk == KT - 2))
                                nc.scalar.activation(q_sb[:, co, :], ps[:], GELU,
                                                     bias=qb_sb[:, co:co + 1], scale=1.0)
                        else:
                            for co in range(8):
                                ps = psA.tile([128, C], F32, tag="mmA")
                                for kk in range(0, KT, 2):
                                    for nch in range(2):
                                        nsl = slice(nch * 512, (nch + 1) * 512)
                                        mm2(ps[:, nsl], wqk_sb[:, co, kk:kk + 2, :],
                                            h_own[:, kk:kk + 2, nsl],
                                            start=(kk == 0), stop=(kk == KT - 2))
                                nc.scalar.activation(q_sb[:, co, :], ps[:], GELU)

                    if i == 0 and not V_H0_OLD:
                        # stage0 produces h chunk 0 first and the first RS
                        # lands only mid-head: front-load all own-row work
                        kv_block(h_own[:, :, 0:512], 0)
                        kv_block(h_own[:, :, 512:1024], 1)
                        q_block()
                        scores(range(8), 0)
                        scores(range(8), 1)
                        att_own_block()
                        partner_kv(0)
                        scores(range(8, 12), 0)
                        scores(range(8, 12), 1)
                        if not XCH1:
                            partner_kv(1)
                        scores(range(12, 16), 0)
                        scores(range(12, 16), 1)
                    else:
                        q_block()
                        kv_block(h_own[:, :, 0:512], 0)
                        kv_block(h_own[:, :, 512:1024], 1)
                        scores(range(8), 0)
                        scores(range(8), 1)
                        att_own_block()
                        partner_kv(0)
                        if not XCH1:
                            partner_kv(1)
                        scores(range(8, 16), 0)
                        scores(range(8, 16), 1)

                    # ---- att^T, channel softmax, proj per s-half

                    for sh in range(2):
                        ssl = slice(sh * 512, (sh + 1) * 512)
                        for cop in range(0, 8, 2):
                            ps = psA.tile([128, C], F32, tag="mmA", name="psatt")
                            for j in range(2):
                                jsl = slice(j * 512, (j + 1) * 512)
                                for tt in range(8, 16, 2):
                                    mm2(ps[:, jsl],
                                        v_sb[:, tt:tt + 2, (cop + j) * 128:(cop + j + 1) * 128],
                                        sc_sb[:, tt:tt + 2, ssl],
                                        start=(tt == 8), stop=False)
                                nc.tensor.matmul(ps[:, jsl], ident_sb[:],
                                                 att_own8[:, cop + j, ssl],
                                                 start=False, stop=True)
                            nc.scalar.activation(attsmT[:, cop:cop + 2, ssl], ps[:], EXP)
                    for sh in range(2):
                        ssl = slice(sh * 512, (sh + 1) * 512)
                        dn = psB.tile([1, 512], F32, tag="denom", bufs=1, name="dn")
                        for co in range(8):
                            nc.tensor.matmul(dn[:], ones_col[:], attsmT[:, co, ssl],
                                             start=(co == 0), stop=(co == 7))
                        rr = small.tile([1, 512], F32, tag="rr", name="rr")
                        nc.vector.reciprocal(rr[:], dn[:])
                        rrb = small.tile([1, 512], BF16, tag="rrb", name="rrb")
                        nc.vector.tensor_copy(rrb[:], rr[:])
                        bc = psB.tile([128, 512], F32, tag="mmB", name="bc")
                        nc.tensor.matmul(bc[:], ones_bf[:], rrb[:], start=True, stop=True)
                        bc_sb = tmp.tile([128, 512], F32, tag="bcsb", name="bc_sb")
                        nc.vector.tensor_copy(bc_sb[:], bc[:])
                        for co in range(8):
                            # split normalize across DVE and the idle GpSimd
                            if co < 5 or (sh == 1 and os.environ.get("V_SM1_DVE", "0") == "1"):
                                nc.vector.tensor_mul(attn8[:, co, ssl],
                                                     attsmT[:, co, ssl], bc[:])
                            else:
                                nc.gpsimd.tensor_mul(attn8[:, co, ssl],
                                                     attsmT[:, co, ssl], bc_sb[:])
                    for sh in range(2):
                        ssl = slice(sh * 512, (sh + 1) * 512)
                        if with_bias:
                            for co in range(8):
                                ps = psB.tile([128, 512], F32, tag="mmB", name="psp")
                                for cc in range(0, 8, 2):
                                    mm2(ps[:], wp_sb[:, co, cc:cc + 2, :],
                                        attn8[:, cc:cc + 2, ssl], start=(cc == 0), stop=(cc == 6))
                                if i == 0:
                                    nc.scalar.activation(h_new[:, co, ssl], ps[:], GELU,
                                                         bias=pb_sb[:, co:co + 1],
                                                         scale=1.0 / SM_SCALE)
                                    nc.vector.tensor_copy(pred[:, co, ssl], h_new[:, co, ssl])
                                else:
                                    gtmp = tmp.tile([128, 512], BF16, tag="gtmp", name="gtmp")
                                    nc.scalar.activation(gtmp[:], ps[:], GELU,
                                                         bias=pb_sb[:, co:co + 1],
                                                         scale=1.0 / SM_SCALE)
                                    nc.vector.tensor_add(h_new[:, co, ssl], gtmp[:],
                                                         pred[:, co, ssl])
                        else:
                            for cop in range(0, 8, 2):
                                ps = psA.tile([128, C], F32, tag="mmA", name="psp")
                                for j in range(2):
                                    jsl = slice(j * 512, (j + 1) * 512)
                                    for cc in range(0, 8, 2):
                                        mm2(ps[:, jsl], wp_sb[:, cop + j, cc:cc + 2, :],
                                            attn8[:, cc:cc + 2, ssl],
                                            start=(cc == 0), stop=(cc == 6))
                                if i == 0:
                                    nc.scalar.activation(h_new[:, cop:cop + 2, ssl], ps[:],
                                                         GELU, scale=1.0 / SM_SCALE)
                                    nc.vector.tensor_copy(pred[:, cop:cop + 2, ssl],
                                                          h_new[:, cop:cop + 2, ssl])
                                else:
                                    gtmp = tmp.tile([128, C], BF16, tag="gtmp", name="gtmp")
                                    nc.scalar.activation(gtmp[:], ps[:], GELU,
                                                         scale=1.0 / SM_SCALE)
                                    # residual adds split across DVE and GpSimd
                                    radd = nc.gpsimd if cop >= 4 else nc.vector
                                    radd.tensor_add(h_new[:, cop:cop + 2, ssl], gtmp[:],
                                                    pred[:, cop:cop + 2, ssl])
                        if i + 1 < n_heads and (not XCH1 or sh == 1):
                            launch_rs(i + 1, h_new, 0 if XCH1 else sh)
                    h_own = h_new

            # ---------------- fc_out + pose encoding ----------------
            with (
                tc.tile_pool(name="fco", bufs=1) as fo,
                tc.tile_pool(name="fco2", bufs=2) as fo2,
            ):
                fow_sb = fo.tile([128, KT, C], F8)
                nc.sync.dma_start(fow_sb[:], fow[:].rearrange("(k p) n -> p k n", p=128))
                pe_sb = fo.tile([128, 8, C], F32)
                nc.sync.dma_start(pe_sb[:], pe[:].rearrange("(s p) c -> p s c", p=128))
                if with_bias:
                    fob_sb = fo.tile([1, C], BF16)
                    nc.sync.dma_start(fob_sb[:], fob[:])
                for ss in range(8):
                    ps = psA.tile([128, C], F32, tag="mmA")
                    for kk in range(0, KT, 2):
                        for nch in range(2):
                            nsl = slice(nch * 512, (nch + 1) * 512)
                            mm2(ps[:, nsl], h_own[:, kk:kk + 2, ss * 128:(ss + 1) * 128],
                                fow_sb[:, kk:kk + 2, nsl], start=(kk == 0),
                                stop=(not with_bias and kk == KT - 2))
                    if with_bias:
                        for nch in range(2):
                            nsl = slice(nch * 512, (nch + 1) * 512)
                            nc.tensor.matmul(ps[:, nsl], ones_bf[:], fob_sb[0:1, nsl],
                                             start=False, stop=True)
                    o_sb = fo2.tile([128, C], F32, tag="osb")
                    nc.vector.tensor_add(o_sb[:], ps[:], pe_sb[:, ss, :])
                    nc.sync.dma_start(out[ss * 128:(ss + 1) * 128, :], o_sb[:])
            psB_cm.__exit__(None, None, None)
            psA_cm.__exit__(None, None, None)

    nc.compile()
    return nc


def build_null() -> bacc.Bacc:
    """Same I/O signature, ~no compute: measures the dispatch floor."""
    nc = bacc.Bacc(num_devices=N_CORES, name="attn_null")
    nc.dram_tensor("x_t", [C, S_OWN], F8, kind="ExternalInput")
    nc.dram_tensor("fc_in_wT", [C, C], F8, kind="ExternalInput")
    nc.dram_tensor("fc_in_b_row", [1, C], BF16, kind="ExternalInput")
    nc.dram_tensor("ln_g_row", [1, C], F32, kind="ExternalInput")
    nc.dram_tensor("ln_b_row", [1, C], F32, kind="ExternalInput")
    nc.dram_tensor("qk_w_tiled", [H, 16, 128, KT, 128], F8, kind="ExternalInput")
    nc.dram_tensor("v_wT", [H, C, C], F8, kind="ExternalInput")
    nc.dram_tensor("q_b_col", [H, 128, 8], F32, kind="ExternalInput")
    nc.dram_tensor("k_b_col", [H, 128, 8], F32, kind="ExternalInput")
    nc.dram_tensor("v_b_row", [H, 1, C], BF16, kind="ExternalInput")
    nc.dram_tensor("proj_w_tiled", [H, 8, 128, KT, 128], F8, kind="ExternalInput")
    nc.dram_tensor("proj_b_col", [H, 128, 8], F32, kind="ExternalInput")
    nc.dram_tensor("fc_out_wT", [C, C], F8, kind="ExternalInput")
    nc.dram_tensor("fc_out_b_row", [1, C], BF16, kind="ExternalInput")
    pe = nc.dram_tensor("pe", [S_OWN, C], F32, kind="ExternalInput")
    nc.dram_tensor("ident128", [128, 128], F8, kind="ExternalInput")
    out = nc.dram_tensor("out", [S_OWN, C], F32, kind="ExternalOutput")
    with tile.TileContext(nc) as tc:
        with tc.tile_pool(name="p", bufs=2) as p:
            for ss in range(8):
                t = p.tile([128, C], F32, tag="t")
                nc.sync.dma_start(t[:], pe[ss * 128:(ss + 1) * 128, :])
                nc.sync.dma_start(out[ss * 128:(ss + 1) * 128, :], t[:])
    nc.compile()
    return nc


def _pose_enc_np(s, f):
    pos = np.arange(s, dtype=np.float32)[:, None]
    div = (1.0 / (1000.0 ** (2.0 * np.arange(f, dtype=np.float32) / np.float32(f))))[None, :]
    p = np.zeros((s, f), np.float32)
    p[0::2, :] = np.sin(pos[0::2] * div)
    p[1::2, :] = np.cos(pos[1::2] * div)
    return p


def _bf(a):
    return np.ascontiguousarray(np.asarray(a, np.float32).astype(NP_BF16))


def _f8(a):
    return np.ascontiguousarray(np.asarray(a, np.float32).astype(NP_F8))


def _f32(a):
    return np.ascontiguousarray(np.asarray(a, np.float32))


def prepare_in_maps(x, fc_in_w, fc_in_b, ln_g, ln_b, qkv_w, qkv_b, proj_w, proj_b,
                    fc_out_w, fc_out_b):
    x = np.asarray(x, np.float32)
    qkv_w = np.asarray(qkv_w, np.float32)
    qkv_b = np.asarray(qkv_b, np.float32)
    proj_w = np.asarray(proj_w, np.float32)

    # [H, c_in, 2C] with q columns then k columns -> [H, 16, 128, KT, 128]
    qkT = np.concatenate([qkv_w[:, 0:C, :].transpose(0, 2, 1),
                          qkv_w[:, C:2 * C, :].transpose(0, 2, 1)], axis=2)
    qk_tiled = _f8(qkT.reshape(H, KT, 128, 16, 128).transpose(0, 3, 2, 1, 4))
    v_wT = _f8(qkv_w[:, 2 * C:, :].transpose(0, 2, 1))
    pw_tiled = _f8(proj_w.transpose(0, 2, 1).reshape(H, KT, 128, 8, 128).transpose(0, 3, 2, 1, 4))

    shared = {
        "fc_in_wT": _f8(np.asarray(fc_in_w, np.float32).T),
        "fc_in_b_row": _bf(np.asarray(fc_in_b)[None, :]),
        "ln_g_row": _f32(np.asarray(ln_g)[None, :]),
        "ln_b_row": _f32(np.asarray(ln_b)[None, :]),
        "qk_w_tiled": qk_tiled,
        "v_wT": v_wT,
        "q_b_col": _f32(qkv_b[:, 0:C].reshape(H, 8, 128).transpose(0, 2, 1)),
        "k_b_col": _f32(qkv_b[:, C:2 * C].reshape(H, 8, 128).transpose(0, 2, 1)),
        "v_b_row": _bf(qkv_b[:, 2 * C:][:, None, :]),
        "proj_w_tiled": pw_tiled,
        "proj_b_col": _f32(np.asarray(proj_b, np.float32).reshape(H, 8, 128).transpose(0, 2, 1)),
        "fc_out_wT": _f8(np.asarray(fc_out_w, np.float32).T),
        "fc_out_b_row": _bf(np.asarray(fc_out_b)[None, :]),
    }
    pe_full = _pose_enc_np(S, C)
    ident128 = _f8(np.eye(128, dtype=np.float32))
    in_maps = []
    for core in range(N_CORES):
        b, half = divmod(core, 2)
        m = dict(shared)
        m["ident128"] = ident128
        m["x_t"] = _f8(x[b, half * S_OWN:(half + 1) * S_OWN, :].T)
        m["pe"] = np.ascontiguousarray(pe_full[half * S_OWN:(half + 1) * S_OWN, :])
        in_maps.append(m)
    return in_maps


_NC_CACHE = {}


def get_nc(n_heads=H, with_bias=True, with_ln_affine=True):
    key = (n_heads, with_bias, with_ln_affine)
    if key not in _NC_CACHE:
        _NC_CACHE[key] = build(n_heads, with_bias, with_ln_affine)
    return _NC_CACHE[key]


_EXEC_CACHE = {}


def _get_executable(nc):
    """One jitted collectives executable per process (loading a second one
    hangs the axon worker); reused across kernel() calls."""
    key = id(nc)
    if key in _EXEC_CACHE:
        return _EXEC_CACHE[key]
    import jax
    from jax.sharding import Mesh, PartitionSpec, NamedSharding
    from jax.experimental.shard_map import shard_map
    from concourse import bass2jax
    import concourse.mybir as mybir_

    bass2jax.install_neuronx_cc_hook()
    partition_name = nc.partition_id_tensor.name if nc.partition_id_tensor else None
    in_names, out_names, out_avals, zero_outs = [], [], [], []
    for alloc in nc.m.functions[0].allocations:
        if not isinstance(alloc, mybir_.MemoryLocationSet):
            continue
        name = alloc.memorylocations[0].name
        if alloc.kind == "ExternalInput":
            if name != partition_name:
                in_names.append(name)
        elif alloc.kind == "ExternalOutput":
            out_names.append(name)
            shape = tuple(alloc.tensor_shape)
            dtype = mybir_.dt.np(alloc.dtype)
            out_avals.append(jax.core.ShapedArray(shape, dtype))
            zero_outs.append(np.zeros(shape, dtype))
    n_params = len(in_names)
    n_outs = len(out_avals)
    all_in = in_names + out_names + ([partition_name] if partition_name else [])
    donate = tuple(range(n_params, n_params + n_outs))

    def _body(*args):
        operands = list(args)
        if partition_name is not None:
            operands.append(bass2jax.partition_id_tensor())
        return tuple(bass2jax._bass_exec_p.bind(
            *operands, out_avals=tuple(out_avals), in_names=tuple(all_in),
            out_names=tuple(out_names), lowering_input_output_aliases=(),
            sim_require_finite=True, sim_require_nnan=True, nc=nc))

    devices = jax.devices()[:N_CORES]
    mesh = Mesh(np.asarray(devices), ("core",))
    sharded = jax.jit(
        shard_map(_body, mesh=mesh,
                  in_specs=(PartitionSpec("core"),) * (n_params + n_outs),
                  out_specs=(PartitionSpec("core"),) * len(out_names),
                  check_rep=False),
        donate_argnums=donate, keep_unused=True)
    sh = NamedSharding(mesh, PartitionSpec("core"))
    entry = (sharded, sh, in_names[:n_params], out_names, out_avals, zero_outs)
    _EXEC_CACHE[key] = entry
    return entry


def flags_for(inputs):
    with_bias = not (np.all(np.asarray(inputs["fc_in_b"]) == 0)
                     and np.all(np.asarray(inputs["qkv_b"]) == 0)
                     and np.all(np.asarray(inputs["proj_b"]) == 0)
                     and np.all(np.asarray(inputs["fc_out_b"]) == 0))
    with_ln = not (np.all(np.asarray(inputs["ln_g"]) == 1)
                   and np.all(np.asarray(inputs["ln_b"]) == 0))
    return with_bias, with_ln


def kernel(**inputs) -> np.ndarray:
    with_bias, with_ln = flags_for(inputs)
    nc = get_nc(H, with_bias, with_ln)
    in_maps = prepare_in_maps(**inputs)
    import jax
    sharded, sh, in_names, out_names, out_avals, zero_outs = _get_executable(nc)
    concat_in = [jax.device_put(
        np.concatenate([np.asarray(in_maps[c][nm]) for c in range(N_CORES)], axis=0), sh)
        for nm in in_names]
    concat_zeros = [jax.device_put(
        np.zeros((N_CORES * z.shape[0], *z.shape[1:]), z.dtype), sh) for z in zero_outs]
    out_arrs = sharded(*concat_in, *concat_zeros)
    jax.block_until_ready(out_arrs)
    oi = out_names.index("out")
    per_core = np.asarray(out_arrs[oi]).reshape(N_CORES, *out_avals[oi].shape)
    out_full = np.empty((B, S, C), np.float32)
    for core in range(N_CORES):
        b, half = divmod(core, 2)
        out_full[b, half * S_OWN:(half + 1) * S_OWN, :] = per_core[core]
    return out_full


# revision 42
# speedup vs baseline: 15.7319x; 1.0161x over previous
"""Distributed Trainium2 kernel for nn_Attention_18562848653411.

Reference model: fc_in -> LayerNorm -> 4 sequential "refinement heads"
(qkv matmul + gelu, scores=q@k^T/C, att=scores@v, softmax over channels,
proj + gelu, residual with head-0 output) -> fc_out + PoseEncoding.

Sharding (8 NeuronCores): core c handles batch b=c//2, sequence half
h=c%2 (1024 of 2048 rows). All weights replicated; every stage is
row-local except k/v, which need h for the full sequence.

Pair exchange runs as a ReduceScatter sum-trick instead of AllGather:
each core sends its h^T chunk DUPLICATED ([h; h], so the collective is
rank-symmetric), receives sum = h_own + h_partner, and recovers the
partner half with subtracts split across DVE and GpSimd. This keeps
the graph SPMD-uniform with the partner data at a fixed t-slot, moves
half the collective bytes of an AllGather, and lets all own-row work
run before the collective lands: q, own-row k/v, own-t scores, AND the
own-t half of the att accumulation, which is staged to fp8 and folded
back into the partner-half PSUM group with an identity matmul.
fc_in+LayerNorm is computed for own rows only (its own 4-deep PSUM
pool, freed before the head pools open) and head 0 consumes the same
exchange as every other head. Per-head weights arrive as three large
prefetched DMAs; elementwise work is spread across DVE/GpSimd/Scalar
(the softmax normalize and residual adds are split between DVE and
GpSimd); gelu/exp activations process merged [128,1024] co-pairs and
all exp calls run back-to-back so each head pays only two activation
table loads.

Activations are kept in transposed [C, S] layout so every matmul
consumes operands natively (contraction on partitions); att^T is
produced directly with v-stationary matmuls and the channel softmax
uses a ones-matmul denominator + rank-1 broadcast (att <= ~10, so exp
needs no max subtraction).

Compute: fp8e4 (e4m3) matmuls with DoubleRow perf mode (2 fp8 weights
per PE cell -> 2 k-tiles per instruction) and f32 PSUM accumulation.
The channel softmax renormalizes away quantization noise each head and
the exact-f32 pose encoding dominates the output, so end-to-end rel
err stays ~3.5e-4 (measured vs the f32 reference; gate is 2e-2). The
softmax probabilities are stored scaled by 64 so they sit in fp8e4's
normal range; the proj activation applies the compensating 1/64 via
its input scale. exp() values (up to e^10) stay bf16. DMAs are batched
into few large multi-dim transfers (the DGE costs ~625ns per
instruction regardless of size).
"""

import numpy as np
import ml_dtypes

import concourse.bass as bass
import concourse.mybir as mybir
import concourse.tile as tile
from concourse import bacc
from concourse.bass_utils import run_bass_kernel_spmd

N_CORES = 8
PAIRS = [[0, 1], [2, 3], [4, 5], [6, 7]]
B, S, C = 4, 2048, 1024
H = 4
S_OWN = S // 2        # rows per core
T = S                 # full sequence (k/v length)
KT = C // 128         # contraction tiles
SH2 = S_OWN // 2      # exchange chunk (s columns)

F32 = mybir.dt.float32
BF16 = mybir.dt.bfloat16
F8 = mybir.dt.float8e4
GELU = mybir.ActivationFunctionType.Gelu
EXP = mybir.ActivationFunctionType.Exp
SQRT = mybir.ActivationFunctionType.Sqrt
IDENT = mybir.ActivationFunctionType.Identity
SUB = mybir.AluOpType.subtract
MULT = mybir.AluOpType.mult
ADD = mybir.AluOpType.add
BYPASS = mybir.AluOpType.bypass
DROW = mybir.MatmulPerfMode.DoubleRow

NP_BF16 = ml_dtypes.bfloat16
NP_F8 = ml_dtypes.float8_e4m3

SM_SCALE = 64.0       # softmax store scale (fp8e4 normal range)

import os
V_NORM_ACT = os.environ.get("V_NORM_ACT", "1") == "1"   # LN normalize on scalar engine
V_TRANS_SP = os.environ.get("V_TRANS_SP", "1") == "1"   # transposes on SP queue
V_SUB_JOINT = os.environ.get("V_SUB_JOINT", "0") == "1" # single Pool subtract
V_H0_OLD = os.environ.get("V_H0_OLD", "0") == "1"       # head0 phase order like other heads


def build(n_heads: int = H, with_bias: bool = True, with_ln_affine: bool = True) -> bacc.Bacc:
    """Build the SPMD graph. n_heads>4 cycles weights (timing builds).
    with_bias=False elides the K=1 bias matmuls (all harness biases are 0);
    with_ln_affine=False elides the LayerNorm gain/bias application."""
    nc = bacc.Bacc(num_devices=N_CORES, name="attn")

    x_t = nc.dram_tensor("x_t", [C, S_OWN], F8, kind="ExternalInput")
    fcw = nc.dram_tensor("fc_in_wT", [C, C], F8, kind="ExternalInput")
    fcb = nc.dram_tensor("fc_in_b_row", [1, C], BF16, kind="ExternalInput")
    lng = nc.dram_tensor("ln_g_row", [1, C], F32, kind="ExternalInput")
    lnb = nc.dram_tensor("ln_b_row", [1, C], F32, kind="ExternalInput")
    qkw = nc.dram_tensor("qk_w_tiled", [H, 16, 128, KT, 128], F8, kind="ExternalInput")
    vw = nc.dram_tensor("v_wT", [H, C, C], F8, kind="ExternalInput")
    qb = nc.dram_tensor("q_b_col", [H, 128, 8], F32, kind="ExternalInput")
    kb = nc.dram_tensor("k_b_col", [H, 128, 8], F32, kind="ExternalInput")
    vb = nc.dram_tensor("v_b_row", [H, 1, C], BF16, kind="ExternalInput")
    pw = nc.dram_tensor("proj_w_tiled", [H, 8, 128, KT, 128], F8, kind="ExternalInput")
    pb = nc.dram_tensor("proj_b_col", [H, 128, 8], F32, kind="ExternalInput")
    fow = nc.dram_tensor("fc_out_wT", [C, C], F8, kind="ExternalInput")
    fob = nc.dram_tensor("fc_out_b_row", [1, C], BF16, kind="ExternalInput")
    pe = nc.dram_tensor("pe", [S_OWN, C], F32, kind="ExternalInput")
    ident = nc.dram_tensor("ident128", [128, 128], F8, kind="ExternalInput")
    out = nc.dram_tensor("out", [S_OWN, C], F32, kind="ExternalOutput")

    def mm2(ps, lhsT, rhs, start, stop):
        nc.tensor.matmul(ps, lhsT, rhs, start=start, stop=stop, perf_mode=DROW)

    with tile.TileContext(nc) as tc:
        with (
            tc.tile_pool(name="dram", bufs=1, space="DRAM") as dram,
            tc.tile_pool(name="pers", bufs=1) as pers,
            tc.tile_pool(name="hown", bufs=2) as hown_pool,
            tc.tile_pool(name="wts", bufs=2) as wts,
            tc.tile_pool(name="small", bufs=2) as small,
            tc.tile_pool(name="tmp", bufs=2) as tmp,
            tc.tile_pool(name="xch", bufs=1 if os.environ.get("V_XCH1", "0") == "1" else 2) as xch,
        ):
            # per-head pair exchange buffers (RS sum trick)
            XCH1 = os.environ.get("V_XCH1", "0") == "1"
            CW = S_OWN if XCH1 else SH2  # exchange chunk width (s columns)
            NCH = 1 if XCH1 else 2
            bounce_in = [[dram.tile([2, C, CW], F8, name=f"xin{i}_{ch}")
                          for ch in range(NCH)] for i in range(n_heads)]
            rs_out = [[dram.tile([C, CW], F8, name=f"xout{i}_{ch}")
                       for ch in range(NCH)] for i in range(n_heads)]

            ones_bf = pers.tile([1, 128], BF16)
            nc.vector.memset(ones_bf[:], 1.0)
            # 1/SM_SCALE so the softmax reciprocal comes out pre-scaled
            ones_col = pers.tile([128, 1], BF16)
            nc.vector.memset(ones_col[:], 1.0 / SM_SCALE)
            eps_t = pers.tile([128, 1], F32)
            nc.vector.memset(eps_t[:], 1e-5)
            pred = pers.tile([128, KT, S_OWN], F8)
            ident_sb = pers.tile([128, 128], F8)
            nc.sync.dma_start(ident_sb[:], ident[:])

            def launch_rs(idx, h_src, ch):
                """Send [h;h] of s-chunk ch, ReduceScatter-add across the pair.
                rs_out = h_own + h_partner for those s columns."""
                csl = slice(ch * CW, (ch + 1) * CW)
                beng = nc.sync if os.environ.get("V_BOUNCE_SP", "1") == "1" else nc.gpsimd
                for dup in range(2):
                    beng.dma_start(
                        bounce_in[idx][ch][dup].rearrange("(k p) s -> p k s", p=128),
                        h_src[:, :, csl])
                nc.gpsimd.collective_compute(
                    "ReduceScatter", ADD, replica_groups=PAIRS,
                    ins=[bounce_in[idx][ch][:].opt()], outs=[rs_out[idx][ch][:].opt()],
                )

            def load_weights_qk(j):
                wqk_sb = wts.tile([128, 16, KT, 128], F8, tag="wqk")
                nc.sync.dma_start(wqk_sb[:], qkw[j % H].rearrange("c p k f -> p c k f"))
                return wqk_sb

            def load_weights_vp(j):
                wj = j % H
                wv_sb = wts.tile([128, KT, C], F8, tag="wv")
                nc.sync.dma_start(wv_sb[:], vw[wj].rearrange("(k p) n -> p k n", p=128))
                wp_sb = wts.tile([128, 8, KT, 128], F8, tag="wp")
                nc.sync.dma_start(wp_sb[:], pw[wj].rearrange("c p k f -> p c k f"))
                return wv_sb, wp_sb

            def load_weights(j):
                wqk_sb = load_weights_qk(j)
                wv_sb, wp_sb = load_weights_vp(j)
                return wqk_sb, wv_sb, wp_sb

            # -------- stage 0: fc_in + LayerNorm over OWN rows only ------
            h_own = hown_pool.tile([128, KT, S_OWN], F8, tag="hown", name="hT0")
            ps0_cm = tc.tile_pool(name="ps0", bufs=4, space="PSUM")
            ps0 = ps0_cm.__enter__()
            with tc.tile_pool(name="stage0", bufs=1) as s0:
                x_sb = s0.tile([128, KT, S_OWN], F8)
                nc.sync.dma_start(x_sb[:, :, 0:512],
                                  x_t[:, 0:512].rearrange("(k p) s -> p k s", p=128))
                fcw_sb = s0.tile([128, KT, C], F8)
                nc.sync.dma_start(fcw_sb[:], fcw[:].rearrange("(k p) n -> p k n", p=128))
                nc.sync.dma_start(x_sb[:, :, 512:1024],
                                  x_t[:, 512:1024].rearrange("(k p) s -> p k s", p=128))
                if with_bias:
                    fcb_sb = s0.tile([1, C], BF16)
                    nc.sync.dma_start(fcb_sb[:], fcb[:])
                if with_ln_affine:
                    g_bc = s0.tile([128, C], F32)
                    nc.sync.dma_start(g_bc[:], bass.AP(tensor=lng, offset=0,
                                                       ap=[[0, 128], [1, C]]))
                    b_bc = s0.tile([128, C], F32)
                    nc.sync.dma_start(b_bc[:], bass.AP(tensor=lnb, offset=0,
                                                       ap=[[0, 128], [1, C]]))
                w_cur = load_weights(0)

                for ss in range(8):
                    ps = ps0.tile([128, C], F32, tag="mm0")
                    for kk in range(0, KT, 2):
                        for nch in range(2):
                            nsl = slice(nch * 512, (nch + 1) * 512)
                            mm2(ps[:, nsl], x_sb[:, kk:kk + 2, ss * 128:(ss + 1) * 128],
                                fcw_sb[:, kk:kk + 2, nsl], start=(kk == 0),
                                stop=(not with_bias and kk == KT - 2))
                    if with_bias:
                        for nch in range(2):
                            nsl = slice(nch * 512, (nch + 1) * 512)
                            nc.tensor.matmul(ps[:, nsl], ones_bf[:], fcb_sb[0:1, nsl],
                                             start=False, stop=True)
                    stats = small.tile([128, 2, 6], F32, tag="bnst")
                    nc.vector.bn_stats(stats[:, 0, :], ps[:, 0:512])
                    nc.vector.bn_stats(stats[:, 1, :], ps[:, 512:1024])
                    mv = small.tile([128, 2], F32, tag="mv")
                    nc.vector.bn_aggr(mv[:], stats[:])
                    rstd = small.tile([128, 1], F32, tag="rstd")
                    nc.scalar.activation(rstd[:], mv[:, 1:2], SQRT, bias=eps_t[:], scale=1.0)
                    nc.vector.reciprocal(rstd[:], rstd[:])
                    hnb = s0.tile([128, C], BF16, tag="hnb", bufs=3)
                    if with_ln_affine:
                        hn = s0.tile([128, C], F32, tag="hn", bufs=2)
                        nc.vector.tensor_scalar(hn[:], ps[:], mv[:, 0:1], rstd[:],
                                                op0=SUB, op1=MULT)
                        nc.vector.tensor_mul(hn[:], hn[:], g_bc[:])
                        nc.vector.tensor_add(hnb[:], hn[:], b_bc[:])
                    elif V_NORM_ACT:
                        # (x - mu)*rstd on the scalar engine: in*rstd + (-mu*rstd)
                        nmu_rs = small.tile([128, 1], F32, tag="nmurs")
                        nc.vector.tensor_scalar(nmu_rs[:], mv[:, 0:1], rstd[:], -1.0,
                                                op0=MULT, op1=MULT)
                        nc.scalar.activation(hnb[:], ps[:], IDENT,
                                             bias=nmu_rs[:], scale=rstd[:])
                    else:
                        # (x - mu)*rstd on the vector engine
                        nc.vector.tensor_scalar(hnb[:], ps[:], mv[:, 0:1], rstd[:],
                                                op0=SUB, op1=MULT)
                    # bf16 transpose staging (DMA transpose needs 2-byte), then
                    # one DVE pass converts the s-block to fp8. Issued from the
                    # Activation DGE queue so stage0's exchange DMAs (SP queue)
                    # don't queue behind transpose dependencies.
                    h_stg = s0.tile([128, KT, 128], BF16, tag="hstg", bufs=4)
                    (nc.sync if V_TRANS_SP else nc.scalar).dma_start(
                        h_stg[:], hnb[:], transpose=True)
                    nc.vector.tensor_copy(h_own[:, :, ss * 128:(ss + 1) * 128], h_stg[:])
                    if ss == 3 and not XCH1:
                        launch_rs(0, h_own, 0)
                if XCH1:
                    launch_rs(0, h_own, 0)
                else:
                    launch_rs(0, h_own, 1)
            ps0_cm.__exit__(None, None, None)
            psA_cm = tc.tile_pool(name="psA", bufs=2, space="PSUM")
            psA = psA_cm.__enter__()
            psB_cm = tc.tile_pool(name="psB", bufs=3, space="PSUM")
            psB = psB_cm.__enter__()

            # ---------------- heads ----------------
            for i in range(n_heads):
                with (
                    tc.tile_pool(name=f"head{i}", bufs=1) as hp,
                    tc.tile_pool(name=f"attT{i}", bufs=1) as attp,
                ):
                    wqk_sb, wv_sb, wp_sb = w_cur
                    if i + 1 < n_heads:
                        w_cur = load_weights(i + 1)
                    q_sb = hp.tile([128, 8, S_OWN], F8, name="q_sb")
                    k_sb = hp.tile([128, 8, T], F8, name="k_sb")
                    v_sb = hp.tile([128, 16, C], F8, name="v_sb")
                    sc_sb = hp.tile([128, 16, S_OWN], F8, name="sc_sb")
                    if with_bias:
                        vb_sb = small.tile([1, C], BF16, tag="vb", bufs=1)
                        nc.sync.dma_start(vb_sb[:], vb[i % H])
                    qb_sb = small.tile([128, 8], F32, tag="qb")
                    nc.sync.dma_start(qb_sb[:], qb[i % H])
                    kb_sb = small.tile([128, 8], F32, tag="kb")
                    nc.sync.dma_start(kb_sb[:], kb[i % H])
                    pb_sb = small.tile([128, 8], F32, tag="pb")
                    nc.sync.dma_start(pb_sb[:], pb[i % H])

                    def kv_block(hh, tloc):
                        """k^T and v for 512 t-rows given their h^T [128,KT,512].
                        tloc: t-tile base (in units of 128 rows) / 4."""
                        tsl = slice(tloc * 512, (tloc + 1) * 512)
                        if with_bias:
                            for co in range(8):
                                ps = psB.tile([128, 512], F32, tag="mmB", name="psk")
                                for kk in range(0, KT, 2):
                                    mm2(ps[:], wqk_sb[:, 8 + co, kk:kk + 2, :],
                                        hh[:, kk:kk + 2, :], start=(kk == 0),
                                        stop=(kk == KT - 2))
                                nc.scalar.activation(k_sb[:, co, tsl], ps[:], GELU,
                                                     bias=kb_sb[:, co:co + 1], scale=1.0)
                        else:
                            # co-pairs share one [128,1024] PSUM + one act call
                            for cop in range(0, 8, 2):
                                ps = psA.tile([128, C], F32, tag="mmA", name="psk")
                                for j in range(2):
                                    jsl = slice(j * 512, (j + 1) * 512)
                                    for kk in range(0, KT, 2):
                                        mm2(ps[:, jsl], wqk_sb[:, 8 + cop + j, kk:kk + 2, :],
                                            hh[:, kk:kk + 2, :], start=(kk == 0),
                                            stop=(kk == KT - 2))
                                nc.scalar.activation(k_sb[:, cop:cop + 2, tsl], ps[:], GELU)
                        for tt in range(4):
                            ps = psA.tile([128, C], F32, tag="mmA")
                            for kk in range(0, KT, 2):
                                for nch in range(2):
                                    nsl = slice(nch * 512, (nch + 1) * 512)
                                    mm2(ps[:, nsl], hh[:, kk:kk + 2, tt * 128:(tt + 1) * 128],
                                        wv_sb[:, kk:kk + 2, nsl], start=(kk == 0),
                                        stop=(not with_bias and kk == KT - 2))
                            if with_bias:
                                for nch in range(2):
                                    nsl = slice(nch * 512, (nch + 1) * 512)
                                    nc.tensor.matmul(ps[:, nsl], ones_bf[:], vb_sb[0:1, nsl],
                                                     start=False, stop=True)
                            nc.scalar.activation(v_sb[:, tloc * 4 + tt, :], ps[:], GELU)

                    def scores(tt_range, sh):
                        """scores^T for t-tiles tt_range into s-half sh."""
                        ssl = slice(sh * 512, (sh + 1) * 512)
                        for tt in tt_range:
                            ps = psB.tile([128, 512], F32, tag="mmB", name="pss")
                            for cc in range(0, 8, 2):
                                mm2(ps[:], k_sb[:, cc:cc + 2, tt * 128:(tt + 1) * 128],
                                    q_sb[:, cc:cc + 2, ssl], start=(cc == 0), stop=(cc == 6))
                            nc.vector.tensor_scalar_mul(sc_sb[:, tt, ssl], ps[:], 1.0 / C)

                    def partner_kv(ch):
                        """Recover partner h for chunk ch from the RS sum and
                        run its k/v. DMA + subtract are split in kk-halves
                        across DVE and GpSimd to shorten the critical tail."""
                        rsum_sb = xch.tile([128, KT, CW], F8, tag="rsum")
                        hp_sb = xch.tile([128, KT, CW], F8, tag="hpart")
                        csl = slice(ch * CW, (ch + 1) * CW)
                        half = KT // 2
                        nc.scalar.dma_start(
                            rsum_sb[:, 0:half, :],
                            rs_out[i][ch][0:half * 128].rearrange("(k p) s -> p k s", p=128))
                        nc.sync.dma_start(
                            rsum_sb[:, half:KT, :],
                            rs_out[i][ch][half * 128:C].rearrange("(k p) s -> p k s", p=128))
                        nc.vector.tensor_sub(hp_sb[:, 0:half, :], rsum_sb[:, 0:half, :],
                                             h_own[:, 0:half, csl])
                        nc.gpsimd.tensor_sub(hp_sb[:, half:KT, :], rsum_sb[:, half:KT, :],
                                             h_own[:, half:KT, csl])
                        if XCH1:
                            kv_block(hp_sb[:, :, 0:512], 2)
                            kv_block(hp_sb[:, :, 512:1024], 3)
                        else:
                            kv_block(hp_sb, 2 + ch)

                    h_new = hown_pool.tile([128, KT, S_OWN], F8, tag="hown", name=f"hT{i + 1}")
                    attsmT = attp.tile([128, KT, S_OWN], BF16, tag="attT", name="attsmT")
                    attn8 = attp.tile([128, KT, S_OWN], F8, tag="attn8", name="attn8")
                    att_own8 = attp.tile([128, KT, S_OWN], F8, tag="attown", name="att_own8")

                    def att_own_block():
                        # partial att over OWN t-tiles (0..7): all inputs local,
                        # runs while the pair exchange is still in flight
                        for sh in range(2):
                            ssl = slice(sh * 512, (sh + 1) * 512)
                            for cop in range(0, 8, 2):
                                ps = psA.tile([128, C], F32, tag="mmA", name="psao")
                                for j in range(2):
                                    jsl = slice(j * 512, (j + 1) * 512)
                                    for tt in range(0, 8, 2):
                                        mm2(ps[:, jsl],
                                            v_sb[:, tt:tt + 2, (cop + j) * 128:(cop + j + 1) * 128],
                                            sc_sb[:, tt:tt + 2, ssl],
                                            start=(tt == 0), stop=(tt == 6))
                                nc.vector.tensor_copy(att_own8[:, cop:cop + 2, ssl], ps[:])

                    # ---- per-head schedule: own-row work (q, k/v, own-t
                    # scores) carries the PE while the pair exchange lands;
                    # partner chunk 0 arrives well before chunk 1, so its k/v
                    # run between them.
                    def q_block():
                        if with_bias:
                            for co in range(8):
                                ps = psA.tile([128, C], F32, tag="mmA")
                                for kk in range(0, KT, 2):
                                    for nch in range(2):
                                        nsl = slice(nch * 512, (nch + 1) * 512)
                                        mm2(ps[:, nsl], wqk_sb[:, co, kk:kk + 2, :],
                                            h_own[:, kk:kk + 2, nsl],
                                            start=(kk == 0), stop=(kk == KT - 2))
                                nc.scalar.activation(q_sb[:, co, :], ps[:], GELU,
                                                     bias=qb_sb[:, co:co + 1], scale=1.0)
                        else:
                            for co in range(8):
                                ps = psA.tile([128, C], F32, tag="mmA")
                                for kk in range(0, KT, 2):
                                    for nch in range(2):
                                        nsl = slice(nch * 512, (nch + 1) * 512)
                                        mm2(ps[:, nsl], wqk_sb[:, co, kk:kk + 2, :],
                                            h_own[:, kk:kk + 2, nsl],
                                            start=(kk == 0), stop=(kk == KT - 2))
                                nc.scalar.activation(q_sb[:, co, :], ps[:], GELU)

                    if i == 0 and not V_H0_OLD:
                        # stage0 produces h chunk 0 first and the first RS
                        # lands only mid-head: front-load all own-row work
                        kv_block(h_own[:, :, 0:512], 0)
                        kv_block(h_own[:, :, 512:1024], 1)
                        q_block()
                        scores(range(8), 0)
                        scores(range(8), 1)
                        att_own_block()
                        partner_kv(0)
                        scores(range(8, 12), 0)
                        scores(range(8, 12), 1)
                        if not XCH1:
                            partner_kv(1)
                        scores(range(12, 16), 0)
                        scores(range(12, 16), 1)
                    else:
                        q_block()
                        kv_block(h_own[:, :, 0:512], 0)
                        kv_block(h_own[:, :, 512:1024], 1)
                        scores(range(8), 0)
                        scores(range(8), 1)
                        att_own_block()
                        partner_kv(0)
                        if not XCH1:
                            partner_kv(1)
                        scores(range(8, 16), 0)
                        scores(range(8, 16), 1)

                    # ---- att^T, channel softmax, proj per s-half

                    for sh in range(2):
                        ssl = slice(sh * 512, (sh + 1) * 512)
                        for cop in range(0, 8, 2):
                            ps = psA.tile([128, C], F32, tag="mmA", name="psatt")
                            for j in range(2):
                                jsl = slice(j * 512, (j + 1) * 512)
                                for tt in range(8, 16, 2):
                                    mm2(ps[:, jsl],
                                        v_sb[:, tt:tt + 2, (cop + j) * 128:(cop + j + 1) * 128],
                                        sc_sb[:, tt:tt + 2, ssl],
                                        start=(tt == 8), stop=False)
                                nc.tensor.matmul(ps[:, jsl], ident_sb[:],
                                                 att_own8[:, cop + j, ssl],
                                                 start=False, stop=True)
                            nc.scalar.activation(attsmT[:, cop:cop + 2, ssl], ps[:], EXP)
                    for sh in range(2):
                        ssl = slice(sh * 512, (sh + 1) * 512)
                        dn = psB.tile([1, 512], F32, tag="denom", bufs=1, name="dn")
                        for co in range(8):
                            nc.tensor.matmul(dn[:], ones_col[:], attsmT[:, co, ssl],
                                             start=(co == 0), stop=(co == 7))
                        rr = small.tile([1, 512], F32, tag="rr", name="rr")
                        nc.vector.reciprocal(rr[:], dn[:])
                        rrb = small.tile([1, 512], BF16, tag="rrb", name="rrb")
                        nc.vector.tensor_copy(rrb[:], rr[:])
                        bc = psB.tile([128, 512], F32, tag="mmB", name="bc")
                        nc.tensor.matmul(bc[:], ones_bf[:], rrb[:], start=True, stop=True)
                        bc_sb = tmp.tile([128, 512], F32, tag="bcsb", name="bc_sb")
                        nc.vector.tensor_copy(bc_sb[:], bc[:])
                        for co in range(8):
                            # split normalize across DVE and the idle GpSimd
                            if co < 5 or (sh == 1 and os.environ.get("V_SM1_DVE", "0") == "1"):
                                nc.vector.tensor_mul(attn8[:, co, ssl],
                                                     attsmT[:, co, ssl], bc[:])
                            else:
                                nc.gpsimd.tensor_mul(attn8[:, co, ssl],
                                                     attsmT[:, co, ssl], bc_sb[:])
                    for sh in range(2):
                        ssl = slice(sh * 512, (sh + 1) * 512)
                        if with_bias:
                            for co in range(8):
                                ps = psB.tile([128, 512], F32, tag="mmB", name="psp")
                                for cc in range(0, 8, 2):
                                    mm2(ps[:], wp_sb[:, co, cc:cc + 2, :],
                                        attn8[:, cc:cc + 2, ssl], start=(cc == 0), stop=(cc == 6))
                                if i == 0:
                                    nc.scalar.activation(h_new[:, co, ssl], ps[:], GELU,
                                                         bias=pb_sb[:, co:co + 1],
                                                         scale=1.0 / SM_SCALE)
                                    nc.vector.tensor_copy(pred[:, co, ssl], h_new[:, co, ssl])
                                else:
                                    gtmp = tmp.tile([128, 512], BF16, tag="gtmp", name="gtmp")
                                    nc.scalar.activation(gtmp[:], ps[:], GELU,
                                                         bias=pb_sb[:, co:co + 1],
                                                         scale=1.0 / SM_SCALE)
                                    nc.vector.tensor_add(h_new[:, co, ssl], gtmp[:],
                                                         pred[:, co, ssl])
                        else:
                            for cop in range(0, 8, 2):
                                ps = psA.tile([128, C], F32, tag="mmA", name="psp")
                                for j in range(2):
                                    jsl = slice(j * 512, (j + 1) * 512)
                                    for cc in range(0, 8, 2):
                                        mm2(ps[:, jsl], wp_sb[:, cop + j, cc:cc + 2, :],
                                            attn8[:, cc:cc + 2, ssl],
                                            start=(cc == 0), stop=(cc == 6))
                                if i == 0:
                                    nc.scalar.activation(h_new[:, cop:cop + 2, ssl], ps[:],
                                                         GELU, scale=1.0 / SM_SCALE)
                                    nc.vector.tensor_copy(pred[:, cop:cop + 2, ssl],
                                                          h_new[:, cop:cop + 2, ssl])
                                else:
                                    gtmp = tmp.tile([128, C], BF16, tag="gtmp", name="gtmp")
                                    nc.scalar.activation(gtmp[:], ps[:], GELU,
                                                         scale=1.0 / SM_SCALE)
                                    # residual adds split across DVE and GpSimd
                                    radd = nc.gpsimd if cop >= 4 else nc.vector
                                    radd.tensor_add(h_new[:, cop:cop + 2, ssl], gtmp[:],
                                                    pred[:, cop:cop + 2, ssl])
                        if i + 1 < n_heads and (not XCH1 or sh == 1):
                            launch_rs(i + 1, h_new, 0 if XCH1 else sh)
                    h_own = h_new

            # ---------------- fc_out + pose encoding ----------------
            with (
                tc.tile_pool(name="fco", bufs=1) as fo,
                tc.tile_pool(name="fco2", bufs=2) as fo2,
            ):
                fow_sb = fo.tile([128, KT, C], F8)
                nc.sync.dma_start(fow_sb[:], fow[:].rearrange("(k p) n -> p k n", p=128))
                pe_sb = fo.tile([128, 8, C], F32)
                nc.sync.dma_start(pe_sb[:], pe[:].rearrange("(s p) c -> p s c", p=128))
                if with_bias:
                    fob_sb = fo.tile([1, C], BF16)
                    nc.sync.dma_start(fob_sb[:], fob[:])
                for ss in range(8):
                    ps = psA.tile([128, C], F32, tag="mmA")
                    for kk in range(0, KT, 2):
                        for nch in range(2):
                            nsl = slice(nch * 512, (nch + 1) * 512)
                            mm2(ps[:, nsl], h_own[:, kk:kk + 2, ss * 128:(ss + 1) * 128],
                                fow_sb[:, kk:kk + 2, nsl], start=(kk == 0),
                                stop=(not with_bias and kk == KT - 2))
                    if with_bias:
                        for nch in range(2):
                            nsl = slice(nch * 512, (nch + 1) * 512)
                            nc.tensor.matmul(ps[:, nsl], ones_bf[:], fob_sb[0:1, nsl],
                                             start=False, stop=True)
                    o_sb = fo2.tile([128, C], F32, tag="osb")
                    nc.vector.tensor_add(o_sb[:], ps[:], pe_sb[:, ss, :])
                    nc.sync.dma_start(out[ss * 128:(ss + 1) * 128, :], o_sb[:])
            psB_cm.__exit__(None, None, None)
            psA_cm.__exit__(None, None, None)

    nc.compile()
    return nc


def build_null() -> bacc.Bacc:
    """Same I/O signature, ~no compute: measures the dispatch floor."""
    nc = bacc.Bacc(num_devices=N_CORES, name="attn_null")
    nc.dram_tensor("x_t", [C, S_OWN], F8, kind="ExternalInput")
    nc.dram_tensor("fc_in_wT", [C, C], F8, kind="ExternalInput")
    nc.dram_tensor("fc_in_b_row", [1, C], BF16, kind="ExternalInput")
    nc.dram_tensor("ln_g_row", [1, C], F32, kind="ExternalInput")
    nc.dram_tensor("ln_b_row", [1, C], F32, kind="ExternalInput")
    nc.dram_tensor("qk_w_tiled", [H, 16, 128, KT, 128], F8, kind="ExternalInput")
    nc.dram_tensor("v_wT", [H, C, C], F8, kind="ExternalInput")
    nc.dram_tensor("q_b_col", [H, 128, 8], F32, kind="ExternalInput")
    nc.dram_tensor("k_b_col", [H, 128, 8], F32, kind="ExternalInput")
    nc.dram_tensor("v_b_row", [H, 1, C], BF16, kind="ExternalInput")
    nc.dram_tensor("proj_w_tiled", [H, 8, 128, KT, 128], F8, kind="ExternalInput")
    nc.dram_tensor("proj_b_col", [H, 128, 8], F32, kind="ExternalInput")
    nc.dram_tensor("fc_out_wT", [C, C], F8, kind="ExternalInput")
    nc.dram_tensor("fc_out_b_row", [1, C], BF16, kind="ExternalInput")
    pe = nc.dram_tensor("pe", [S_OWN, C], F32, kind="ExternalInput")
    nc.dram_tensor("ident128", [128, 128], F8, kind="ExternalInput")
    out = nc.dram_tensor("out", [S_OWN, C], F32, kind="ExternalOutput")
    with tile.TileContext(nc) as tc:
        with tc.tile_pool(name="p", bufs=2) as p:
            for ss in range(8):
                t = p.tile([128, C], F32, tag="t")
                nc.sync.dma_start(t[:], pe[ss * 128:(ss + 1) * 128, :])
                nc.sync.dma_start(out[ss * 128:(ss + 1) * 128, :], t[:])
    nc.compile()
    return nc


def _pose_enc_np(s, f):
    pos = np.arange(s, dtype=np.float32)[:, None]
    div = (1.0 / (1000.0 ** (2.0 * np.arange(f, dtype=np.float32) / np.float32(f))))[None, :]
    p = np.zeros((s, f), np.float32)
    p[0::2, :] = np.sin(pos[0::2] * div)
    p[1::2, :] = np.cos(pos[1::2] * div)
    return p


def _bf(a):
    return np.ascontiguousarray(np.asarray(a, np.float32).astype(NP_BF16))


def _f8(a):
    return np.ascontiguousarray(np.asarray(a, np.float32).astype(NP_F8))


def _f32(a):
    return np.ascontiguousarray(np.asarray(a, np.float32))


def prepare_in_maps(x, fc_in_w, fc_in_b, ln_g, ln_b, qkv_w, qkv_b, proj_w, proj_b,
                    fc_out_w, fc_out_b):
    x = np.asarray(x, np.float32)
    qkv_w = np.asarray(qkv_w, np.float32)
    qkv_b = np.asarray(qkv_b, np.float32)
    proj_w = np.asarray(proj_w, np.float32)

    # [H, c_in, 2C] with q columns then k columns -> [H, 16, 128, KT, 128]
    qkT = np.concatenate([qkv_w[:, 0:C, :].transpose(0, 2, 1),
                          qkv_w[:, C:2 * C, :].transpose(0, 2, 1)], axis=2)
    qk_tiled = _f8(qkT.reshape(H, KT, 128, 16, 128).transpose(0, 3, 2, 1, 4))
    v_wT = _f8(qkv_w[:, 2 * C:, :].transpose(0, 2, 1))
    pw_tiled = _f8(proj_w.transpose(0, 2, 1).reshape(H, KT, 128, 8, 128).transpose(0, 3, 2, 1, 4))

    shared = {
        "fc_in_wT": _f8(np.asarray(fc_in_w, np.float32).T),
        "fc_in_b_row": _bf(np.asarray(fc_in_b)[None, :]),
        "ln_g_row": _f32(np.asarray(ln_g)[None, :]),
        "ln_b_row": _f32(np.asarray(ln_b)[None, :]),
        "qk_w_tiled": qk_tiled,
        "v_wT": v_wT,
        "q_b_col": _f32(qkv_b[:, 0:C].reshape(H, 8, 128).transpose(0, 2, 1)),
        "k_b_col": _f32(qkv_b[:, C:2 * C].reshape(H, 8, 128).transpose(0, 2, 1)),
        "v_b_row": _bf(qkv_b[:, 2 * C:][:, None, :]),
        "proj_w_tiled": pw_tiled,
        "proj_b_col": _f32(np.asarray(proj_b, np.float32).reshape(H, 8, 128).transpose(0, 2, 1)),
        "fc_out_wT": _f8(np.asarray(fc_out_w, np.float32).T),
        "fc_out_b_row": _bf(np.asarray(fc_out_b)[None, :]),
    }
    pe_full = _pose_enc_np(S, C)
    ident128 = _f8(np.eye(128, dtype=np.float32))
    in_maps = []
    for core in range(N_CORES):
        b, half = divmod(core, 2)
        m = dict(shared)
        m["ident128"] = ident128
        m["x_t"] = _f8(x[b, half * S_OWN:(half + 1) * S_OWN, :].T)
        m["pe"] = np.ascontiguousarray(pe_full[half * S_OWN:(half + 1) * S_OWN, :])
        in_maps.append(m)
    return in_maps


_NC_CACHE = {}


def get_nc(n_heads=H, with_bias=True, with_ln_affine=True):
    key = (n_heads, with_bias, with_ln_affine)
    if key not in _NC_CACHE:
        _NC_CACHE[key] = build(n_heads, with_bias, with_ln_affine)
    return _NC_CACHE[key]


_EXEC_CACHE = {}


def _get_executable(nc):
    """One jitted collectives executable per process (loading a second one
    hangs the axon worker); reused across kernel() calls."""
    key = id(nc)
    if key in _EXEC_CACHE:
        return _EXEC_CACHE[key]
    import jax
    from jax.sharding import Mesh, PartitionSpec, NamedSharding
    from jax.experimental.shard_map import shard_map
    from concourse import bass2jax
    import concourse.mybir as mybir_

    bass2jax.install_neuronx_cc_hook()
    partition_name = nc.partition_id_tensor.name if nc.partition_id_tensor else None
    in_names, out_names, out_avals, zero_outs = [], [], [], []
    for alloc in nc.m.functions[0].allocations:
        if not isinstance(alloc, mybir_.MemoryLocationSet):
            continue
        name = alloc.memorylocations[0].name
        if alloc.kind == "ExternalInput":
            if name != partition_name:
                in_names.append(name)
        elif alloc.kind == "ExternalOutput":
            out_names.append(name)
            shape = tuple(alloc.tensor_shape)
            dtype = mybir_.dt.np(alloc.dtype)
            out_avals.append(jax.core.ShapedArray(shape, dtype))
            zero_outs.append(np.zeros(shape, dtype))
    n_params = len(in_names)
    n_outs = len(out_avals)
    all_in = in_names + out_names + ([partition_name] if partition_name else [])
    donate = tuple(range(n_params, n_params + n_outs))

    def _body(*args):
        operands = list(args)
        if partition_name is not None:
            operands.append(bass2jax.partition_id_tensor())
        return tuple(bass2jax._bass_exec_p.bind(
            *operands, out_avals=tuple(out_avals), in_names=tuple(all_in),
            out_names=tuple(out_names), lowering_input_output_aliases=(),
            sim_require_finite=True, sim_require_nnan=True, nc=nc))

    devices = jax.devices()[:N_CORES]
    mesh = Mesh(np.asarray(devices), ("core",))
    sharded = jax.jit(
        shard_map(_body, mesh=mesh,
                  in_specs=(PartitionSpec("core"),) * (n_params + n_outs),
                  out_specs=(PartitionSpec("core"),) * len(out_names),
                  check_rep=False),
        donate_argnums=donate, keep_unused=True)
    sh = NamedSharding(mesh, PartitionSpec("core"))
    entry = (sharded, sh, in_names[:n_params], out_names, out_avals, zero_outs)
    _EXEC_CACHE[key] = entry
    return entry


def flags_for(inputs):
    with_bias = not (np.all(np.asarray(inputs["fc_in_b"]) == 0)
                     and np.all(np.asarray(inputs["qkv_b"]) == 0)
                     and np.all(np.asarray(inputs["proj_b"]) == 0)
                     and np.all(np.asarray(inputs["fc_out_b"]) == 0))
    with_ln = not (np.all(np.asarray(inputs["ln_g"]) == 1)
                   and np.all(np.asarray(inputs["ln_b"]) == 0))
    return with_bias, with_ln


def kernel(**inputs) -> np.ndarray:
    with_bias, with_ln = flags_for(inputs)
    nc = get_nc(H, with_bias, with_ln)
    in_maps = prepare_in_maps(**inputs)
    import jax
    sharded, sh, in_names, out_names, out_avals, zero_outs = _get_executable(nc)
    concat_in = [jax.device_put(
        np.concatenate([np.asarray(in_maps[c][nm]) for c in range(N_CORES)], axis=0), sh)
        for nm in in_names]
    concat_zeros = [jax.device_put(
        np.zeros((N_CORES * z.shape[0], *z.shape[1:]), z.dtype), sh) for z in zero_outs]
    out_arrs = sharded(*concat_in, *concat_zeros)
    jax.block_until_ready(out_arrs)
    oi = out_names.index("out")
    per_core = np.asarray(out_arrs[oi]).reshape(N_CORES, *out_avals[oi].shape)
    out_full = np.empty((B, S, C), np.float32)
    for core in range(N_CORES):
        b, half = divmod(core, 2)
        out_full[b, half * S_OWN:(half + 1) * S_OWN, :] = per_core[core]
    return out_full
